# revision 1
# baseline (speedup 1.0000x reference)
"""Trainium2 Bass kernel for batched GCN (2x GCNConv + circular Conv1d).

Math per graph (N=64 nodes, S=96 feats, H=512 hidden, E=512 edges):
    deg[d]   = #edges with dst=d (incl. self loop)
    A        = Dinv @ (M0 + I).T @ Dinv,  Dinv = diag(1/sqrt(deg))
    h1       = relu(A @ (x.T @ W1.T) + b1)
    h2       = A @ (h1 @ W2.T) + b2
    y        = circular_conv1d(h2, conv_w).T          # [96, 512]

Device strategy (per core: 64 graphs, processed as 32 pairs of 2 graphs
occupying partition halves 0-63 / 64-127):
  - M0.T built per graph with one-hot matmuls: onehots from a single DVE
    is_equal against an iota table (broadcast APs), then 4 K=128 matmuls
    + identity matmul accumulate counts in PSUM.
  - Row-scale M0.T by dinv (DVE), block-diagonal pair tile, one PE
    transpose -> block-diag Ms = (M_aug @ Dinv) for the pair.
  - agg1 computed transposed (H on partitions) to feed gcn2 without extra
    transposes; dinv folded into z1/z2 copies; bf16 matmuls (fp32 PSUM).
  - conv done as 3 shifted-tap matmuls per graph on a duplicated [h2|h2]
    tile; output lands [l, o]-major in PSUM, copied once, DMA'd out.
"""

import numpy as np
import ml_dtypes

import concourse.bacc as bacc
import concourse.mybir as mybir
import concourse.tile as tile
from concourse.bass_utils import run_bass_kernel_spmd

BF16 = mybir.dt.bfloat16
FP32 = mybir.dt.float32
I32 = mybir.dt.int32
AF = mybir.ActivationFunctionType

N_CORES = 8
B, S, N, H, E = 512, 96, 64, 512, 512
G = B // N_CORES          # graphs per core
NPAIR = G // 2


def build_gcn_kernel(tc, outs, ins, g_per_core=G, has_b1=False, has_b2=False):
    """Emit the kernel into TileContext tc. outs/ins are dicts of DRAM APs."""
    nc = tc.nc
    g = g_per_core
    npair = g // 2

    x_d = ins["x"]          # [g, 96, 64] f32
    ei_d = ins["ei"]        # [2g, 512] i32   (row = 2*graph + (0:src,1:dst))
    w1t_d = ins["w1t"]      # [96, 512] bf16
    w2t_d = ins["w2t"]      # [128, 384] bf16 (f=(c,s))
    cwd_d = ins["cwd"]      # [128, 1536] bf16 (rows 0-63 = [i,(k,o)], dup)
    iota_d = ins["iota"]    # [128, 1024] bf16 (f%64)
    i64d_d = ins["i64d"]    # [128, 64] bf16 (I64 stacked twice)
    id128_d = ins["id128"]  # [128, 128] bf16
    y_d = outs["y"]         # [g, 96, 512] f32

    from contextlib import ExitStack
    ctx = ExitStack()
    const = ctx.enter_context(tc.tile_pool(name="const", bufs=1))
    sb = ctx.enter_context(tc.tile_pool(name="sb", bufs=6))
    psa = ctx.enter_context(tc.tile_pool(name="psa", bufs=3, space="PSUM"))
    ps = ctx.enter_context(tc.tile_pool(name="ps", bufs=2, space="PSUM"))
    ps1 = ctx.enter_context(tc.tile_pool(name="ps1", bufs=1, space="PSUM"))
    psy = ctx.enter_context(tc.tile_pool(name="psy", bufs=2, space="PSUM"))

    # ---- constants ----
    w1t = const.tile([96, 512], BF16)
    nc.sync.dma_start(out=w1t[:], in_=w1t_d[:])
    w2t = const.tile([128, 384], BF16)
    nc.sync.dma_start(out=w2t[:], in_=w2t_d[:])
    cwd = const.tile([128, 1536], BF16)
    nc.sync.dma_start(out=cwd[:], in_=cwd_d[:])
    iota = const.tile([128, 1024], BF16)
    nc.sync.dma_start(out=iota[:], in_=iota_d[:])
    i64d = const.tile([128, 64], BF16)
    nc.sync.dma_start(out=i64d[:], in_=i64d_d[:])
    id128 = const.tile([128, 128], BF16)
    nc.sync.dma_start(out=id128[:], in_=id128_d[:])
    id128f = const.tile([128, 128], FP32)
    nc.vector.tensor_copy(out=id128f[:], in_=id128[:])
    if has_b1:
        b1c = const.tile([128, 4], FP32)
        nc.sync.dma_start(out=b1c[:], in_=ins["b1c"][:])
    if has_b2:
        b2d = const.tile([128, 192], BF16)
        nc.sync.dma_start(out=b2d[:], in_=ins["b2d"][:])

    # ---- x: load + cast to bf16, laid out [s, (g, n)] ----
    xf = const.tile([96, 64 * g], FP32)
    nc.sync.dma_start(out=xf[:].rearrange("s (g n) -> s g n", g=g),
                      in_=x_d[:].rearrange("g s n -> s g n"))
    xbf = const.tile([96, 64 * g], BF16)
    nc.vector.tensor_copy(out=xbf[:], in_=xf[:])

    # ---- edges: load, cast, transpose to [epos, (c, gt)] ----
    ei = const.tile([2 * g, 512], I32)
    nc.sync.dma_start(out=ei[:], in_=ei_d[:])
    eibf = const.tile([2 * g, 512], BF16)
    nc.vector.tensor_copy(out=eibf[:], in_=ei[:])
    et = const.tile([128, 4 * 2 * g], BF16)   # f = (c, gt)
    for c in range(4):
        etp = ps.tile([128, 128], BF16, tag="z1")
        # in_ is [2g, 128] -> out = in_.T = [128, 2g]
        nc.tensor.transpose(
            out=etp[:, 0:2 * g], in_=eibf[:, c * 128:(c + 1) * 128],
            identity=id128[0:2 * g, 0:2 * g],
        )
        nc.scalar.activation(
            out=et[:, c * 2 * g:(c + 1) * 2 * g], in_=etp[:, 0:2 * g],
            func=AF.Copy,
        )

    for pr in range(npair):
        # ---- z1 = x^T W1^T for the pair (independent of A-chain) ----
        z1_ps = ps.tile([128, 512], FP32, tag="z1")
        nc.tensor.matmul(z1_ps[:], xbf[:, 128 * pr:128 * (pr + 1)], w1t[:],
                         start=True, stop=True)

        # ---- one-hots: oh[p, (c, j, v)] = (et[p, (c, 4pr+j)] == v) ----
        e_sl = et[:].rearrange("p (c gt) -> p c gt", c=4)
        e_sl = e_sl[:, :, 4 * pr:4 * pr + 4]
        e_sl = e_sl.rearrange("p c (j u) -> p c j u", u=1)
        e_bc = e_sl.to_broadcast([128, 4, 4, 64])
        erep = sb.tile([128, 1024], BF16, tag="erep")
        nc.gpsimd.tensor_copy(
            out=erep[:].rearrange("p (c j v) -> p c j v", c=4, j=4),
            in_=e_bc)
        oh = sb.tile([128, 1024], BF16, tag="oh")
        nc.vector.tensor_tensor(
            out=oh[:], in0=erep[:], in1=iota[:],
            op=mybir.AluOpType.is_equal,
        )

        # ---- M_aug^T (counts + I) per graph into pair psum [128, 64] ----
        mps = psa.tile([128, 384], FP32, tag="mzz")
        maug = mps[:, 0:64]
        for gl in range(2):
            po = 64 * gl
            out_sl = maug[po:po + 64, :]  # noqa
            tp = None if gl == 0 else (0, 64)
            for c in range(4):
                base = c * 256
                lhsT = oh[:, base + (2 * gl + 1) * 64: base + (2 * gl + 2) * 64]
                rhs = oh[:, base + (2 * gl) * 64: base + (2 * gl + 1) * 64]
                nc.tensor.matmul(out_sl, lhsT, rhs, start=(c == 0),
                                 stop=False, tile_position=tp)
            nc.tensor.matmul(
                out_sl, i64d[po:po + 64, :], i64d[po:po + 64, :],
                start=False, stop=True,
                tile_position=None if gl == 0 else (64, 64),
            )

        # ---- deg -> dinv ----
        deg = sb.tile([128, 1], FP32, tag="deg")
        nc.vector.tensor_reduce(out=deg[:], in_=maug[:, :],
                                axis=mybir.AxisListType.X,
                                op=mybir.AluOpType.add)
        sq = sb.tile([128, 1], FP32, tag="sq")
        nc.scalar.activation(out=sq[:], in_=deg[:], func=AF.Sqrt)
        dinv = sb.tile([128, 1], FP32, tag="dinv")
        nc.vector.reciprocal(out=dinv[:], in_=sq[:])

        # ---- MsT block-diag -> transpose -> Ms block-diag (bf16) ----
        msb = sb.tile([128, 128], FP32, tag="msb")
        nc.gpsimd.memset(msb[:], 0)
        nc.vector.tensor_scalar(
            out=msb[0:64, 0:64], in0=maug[0:64, :], scalar1=dinv[0:64, :],
            scalar2=None, op0=mybir.AluOpType.mult)
        nc.scalar.activation(
            out=msb[64:128, 64:128], in_=maug[64:128, :], func=AF.Copy,
            scale=dinv[64:128, :])
        mst_ps = mps[:, 64:192]
        nc.tensor.transpose(out=mst_ps, in_=msb[:], identity=id128f[:])
        msbd = sb.tile([128, 128], BF16, tag="msbd")
        nc.scalar.activation(out=msbd[:], in_=mst_ps, func=AF.Copy)

        # ---- z1s = dinv * z1 ----
        z1s = sb.tile([128, 512], BF16, tag="z1s")
        nc.vector.tensor_scalar(out=z1s[:], in0=z1_ps[:], scalar1=dinv[:, :],
                                scalar2=None, op0=mybir.AluOpType.mult)

        # ---- agg1T: [128 (h in chunk), (c, g, n)] ----
        a1t_ps = ps1.tile([128, 512], FP32, tag="a1t")
        for c in range(4):
            nc.tensor.matmul(a1t_ps[:, 128 * c:128 * (c + 1)],
                             z1s[:, 128 * c:128 * (c + 1)], msbd[:],
                             start=True, stop=True)
        h1t = sb.tile([128, 512], BF16, tag="h1t")
        if has_b1:
            for c in range(4):
                nc.scalar.activation(
                    out=h1t[:, 128 * c:128 * (c + 1)],
                    in_=a1t_ps[:, 128 * c:128 * (c + 1)],
                    func=AF.Relu, bias=b1c[:, c:c + 1])
        else:
            nc.vector.tensor_scalar_max(h1t[:], a1t_ps[:], 0.0)

        # ---- z2 = h1 W2^T: [128 (g,n), 96 (s)] ----
        z2_ps = mps[:, 192:288]
        for c in range(4):
            nc.tensor.matmul(z2_ps, h1t[:, 128 * c:128 * (c + 1)],
                             w2t[:, 96 * c:96 * (c + 1)],
                             start=(c == 0), stop=(c == 3))
        z2s = sb.tile([128, 96], BF16, tag="z2s")
        nc.scalar.activation(out=z2s[:], in_=z2_ps, func=AF.Copy,
                             scale=dinv[:, :])

        # ---- agg2: [128 (g,n), 96 (l)] ----
        a2_ps = mps[:, 288:384]
        nc.tensor.matmul(a2_ps, msbd[:], z2s[:], start=True, stop=True)

        # ---- h2 duplicated [h2|h2] (+b2); a2 is already fully aggregated ----
        hp = sb.tile([128, 192], BF16, tag="hp")
        nc.vector.tensor_copy(out=hp[:, 0:96], in_=a2_ps)
        nc.scalar.activation(out=hp[:, 96:192], in_=a2_ps, func=AF.Copy)
        if has_b2:
            hpb = sb.tile([128, 192], BF16, tag="hpb")
            nc.vector.tensor_tensor(out=hpb[:], in0=hp[:], in1=b2d[:],
                                    op=mybir.AluOpType.add)
            hp = hpb

        # ---- conv: per graph 3 shifted-tap matmuls -> [96 (l), 512 (o)] ----
        for gl in range(2):
            po = 64 * gl
            y_ps = psy.tile([96, 512], FP32, tag="y")
            for k in range(3):
                tap = (95, 0, 1)[k]
                nc.tensor.matmul(
                    y_ps[:],
                    hp[po:po + 64, tap:tap + 96],
                    cwd[po:po + 64, 512 * k:512 * (k + 1)],
                    start=(k == 0), stop=(k == 2))
            ysb = sb.tile([96, 512], FP32, tag="ysb")
            if gl == 0:
                nc.vector.tensor_copy(out=ysb[:], in_=y_ps[:])
            else:
                nc.scalar.activation(out=ysb[:], in_=y_ps[:], func=AF.Copy)
            nc.sync.dma_start(out=y_d[2 * pr + gl], in_=ysb[:])

    ctx.close()


# ---------------- host side ----------------

def _prep_consts(W1, b1, W2, b2, conv_w):
    bf = ml_dtypes.bfloat16
    w1t = np.ascontiguousarray(W1.T).astype(bf)                    # [96, 512]
    w2t = np.ascontiguousarray(
        W2.T.reshape(4, 128, 96).transpose(1, 0, 2).reshape(128, 384)
    ).astype(bf)
    base = np.ascontiguousarray(conv_w.transpose(1, 2, 0)).reshape(64, 1536)
    cwd = np.concatenate([base, base], axis=0).astype(bf)          # [128, 1536]
    iota = np.broadcast_to((np.arange(1024) % 64).astype(bf), (128, 1024))
    iota = np.ascontiguousarray(iota)
    i64d = np.concatenate([np.eye(64), np.eye(64)], axis=0).astype(bf)
    id128 = np.eye(128).astype(bf)
    consts = dict(w1t=w1t, w2t=w2t, cwd=cwd, iota=iota, i64d=i64d,
                  id128=id128)
    has_b1 = bool(np.any(b1))
    has_b2 = bool(np.any(b2))
    if has_b1:
        consts["b1c"] = np.ascontiguousarray(
            b1.reshape(4, 128).T).astype(np.float32)
    if has_b2:
        b2d = np.ascontiguousarray(
            np.broadcast_to(np.tile(b2, 2).astype(bf), (128, 192)))
        consts["b2d"] = b2d
    return consts, has_b1, has_b2


_NC_CACHE = {}


def _get_nc(g_per_core, has_b1, has_b2):
    key = (g_per_core, has_b1, has_b2)
    if key in _NC_CACHE:
        return _NC_CACHE[key]
    nc = bacc.Bacc("TRN2", target_bir_lowering=False, debug=False)
    ins = {
        "x": nc.dram_tensor("x", [g_per_core, 96, 64], FP32,
                            kind="ExternalInput").ap(),
        "ei": nc.dram_tensor("ei", [2 * g_per_core, 512], I32,
                             kind="ExternalInput").ap(),
        "w1t": nc.dram_tensor("w1t", [96, 512], BF16,
                              kind="ExternalInput").ap(),
        "w2t": nc.dram_tensor("w2t", [128, 384], BF16,
                              kind="ExternalInput").ap(),
        "cwd": nc.dram_tensor("cwd", [128, 1536], BF16,
                              kind="ExternalInput").ap(),
        "iota": nc.dram_tensor("iota", [128, 1024], BF16,
                               kind="ExternalInput").ap(),
        "i64d": nc.dram_tensor("i64d", [128, 64], BF16,
                               kind="ExternalInput").ap(),
        "id128": nc.dram_tensor("id128", [128, 128], BF16,
                                kind="ExternalInput").ap(),
    }
    if has_b1:
        ins["b1c"] = nc.dram_tensor("b1c", [128, 4], FP32,
                                    kind="ExternalInput").ap()
    if has_b2:
        ins["b2d"] = nc.dram_tensor("b2d", [128, 192], BF16,
                                    kind="ExternalInput").ap()
    outs = {
        "y": nc.dram_tensor("y", [g_per_core, 96, 512], FP32,
                            kind="ExternalOutput").ap(),
    }
    with tile.TileContext(nc) as tc:
        build_gcn_kernel(tc, outs, ins, g_per_core, has_b1, has_b2)
    nc.compile()
    _NC_CACHE[key] = nc
    return nc


def kernel(x, edge_index, W1, b1, W2, b2, conv_w, _trace=False):
    x = np.asarray(x)
    edge_index = np.asarray(edge_index)
    consts, has_b1, has_b2 = _prep_consts(
        np.asarray(W1), np.asarray(b1), np.asarray(W2), np.asarray(b2),
        np.asarray(conv_w))
    nc = _get_nc(G, has_b1, has_b2)

    bfcast = {k: v for k, v in consts.items()}
    in_maps = []
    for c in range(N_CORES):
        sl = slice(c * G, (c + 1) * G)
        m = dict(bfcast)
        m["x"] = np.ascontiguousarray(x[sl]).astype(np.float32)
        m["ei"] = np.ascontiguousarray(
            edge_index[sl].reshape(2 * G, 512)).astype(np.int32)
        in_maps.append(m)

    res = run_bass_kernel_spmd(nc, in_maps, core_ids=list(range(N_CORES)),
                               trace=_trace)
    y = np.concatenate([res.results[c]["y"] for c in range(N_CORES)], axis=0)
    if _trace:
        kernel.last_results = res
    return y



# revision 24
# speedup vs baseline: 1.1456x; 1.1456x over previous
"""Trainium2 Bass kernel for batched GCN (2x GCNConv + circular Conv1d).

Math per graph (N=64 nodes, S=96 feats, H=512 hidden, E=512 edges):
    deg[d]   = indegree + 1 (self loop)
    As       = Dinv (C + I) Dinv,  Dinv = diag(1/sqrt(deg)), C[d,s] counts
    h1       = relu((As X) W1^T + b1)          # aggregate-first (96-wide)
    h2       = As (h1 W2^T) + b2
    y        = circular_conv1d(h2, conv_w)     # emitted [o, l]-major

Device strategy (per core: 64 graphs = 32 pairs; pair nodes occupy
partition halves 0-63 / 64-127; pairs processed in groups of 4 to
amortize per-instruction init overheads on the elementwise engines):
  - edges host-transposed to [epos, (pair, chunk, j)] bf16; one-hots
    built by is_equal against an iota table with broadcast-input APs
    (src half on Pool/gpsimd, dst half on DVE) - no materialized
    broadcast copy.
  - C built per graph with K=128 one-hot matmuls + identity matmul
    (tile_position quadrants); deg via one batched reduce per group.
  - As assembled as [s, d]: row-scale by dinv_d, PE transpose (bf16 in,
    fp32 out, 1 cyc/row), row-scale by dinv_s; both GCN normalizations
    live in the matrix - no per-layer rescales.
  - layer1 aggregates x first (96-wide), then expands through W1 chunks
    transposed so layer2 needs no transposes.
  - conv as 12 96-col matmuls per graph ([o,l]-major output); output
    staged bf16, one DMA per pair; host undoes the layout + casts f32.
"""

import numpy as np
import ml_dtypes

import concourse.bacc as bacc
import concourse.mybir as mybir
import concourse.tile as tile
from concourse.bass_utils import run_bass_kernel_spmd

BF16 = mybir.dt.bfloat16
FP32 = mybir.dt.float32
AF = mybir.ActivationFunctionType
MUL = mybir.AluOpType.mult
ISEQ = mybir.AluOpType.is_equal

N_CORES = 8
B, S, N, H, E = 512, 96, 64, 512, 512
G = B // N_CORES          # graphs per core (64)
NPAIR = G // 2            # 32
GRP = 4                   # pairs per group
NGRP = NPAIR // GRP       # 8


def build_gcn_kernel(tc, outs, ins, has_b1=False, has_b2=False):
    nc = tc.nc

    xt_d = ins["xt"]        # [128, 32*96]  bf16  (r n) -> (q s)
    etr_d = ins["etr"]      # [128, 512]    bf16  epos -> (pr, c, j)
    w1t_d = ins["w1t"]      # [96, 512]     bf16  W1.T
    w2t_d = ins["w2t"]      # [128, 384]    bf16  (h_lo) -> (c, s)
    cwd_d = ins["cwd"]      # [128, 1536]   bf16  (dup i) -> (k, oc, o_lo)
    i64d_d = ins["i64d"]    # [128, 64]     bf16  I64 stacked twice
    iota_d = ins["iota"]    # [128, 64]     bf16  rows = 0..63
    y_d = outs["y"]         # [32, 128, 768] bf16

    from contextlib import ExitStack
    ctx = ExitStack()
    const = ctx.enter_context(tc.tile_pool(name="const", bufs=1))
    ohp = ctx.enter_context(tc.tile_pool(name="ohp", bufs=2))
    gsb = ctx.enter_context(tc.tile_pool(name="gsb", bufs=2))
    psb = ctx.enter_context(tc.tile_pool(name="psb", bufs=3))
    gps = ctx.enter_context(tc.tile_pool(name="gps", bufs=1, space="PSUM"))
    tps = ctx.enter_context(tc.tile_pool(name="tps", bufs=1, space="PSUM"))
    pps = ctx.enter_context(tc.tile_pool(name="pps", bufs=1, space="PSUM"))
    cps = ctx.enter_context(tc.tile_pool(name="cps", bufs=2, space="PSUM"))

    # ---- constants ----
    xt = const.tile([128, 32 * 96], BF16)
    nc.sync.dma_start(out=xt[:], in_=xt_d[:])
    etr = const.tile([128, 512], BF16)
    nc.sync.dma_start(out=etr[:], in_=etr_d[:])
    w1t = const.tile([96, 512], BF16)
    nc.sync.dma_start(out=w1t[:], in_=w1t_d[:])
    w2t = const.tile([128, 384], BF16)
    nc.sync.dma_start(out=w2t[:], in_=w2t_d[:])
    cwd = const.tile([128, 1536], BF16)
    nc.sync.dma_start(out=cwd[:], in_=cwd_d[:])
    i64d = const.tile([128, 64], BF16)
    nc.sync.dma_start(out=i64d[:], in_=i64d_d[:])
    iota = const.tile([128, 64], BF16)
    nc.sync.dma_start(out=iota[:], in_=iota_d[:])
    id1f = const.tile([128, 128], FP32)
    nc.sync.dma_start(out=id1f[:], in_=ins["id1f"][:])
    if has_b1:
        b1c = const.tile([128, 4], FP32)
        nc.sync.dma_start(out=b1c[:], in_=ins["b1c"][:])
    if has_b2:
        b2d = const.tile([128, 192], BF16)
        nc.sync.dma_start(out=b2d[:], in_=ins["b2d"][:])

    # per-(pair,chunk) col layout inside oh:
    #   [src0 64 | src1 64 | dst0 64 | dst1 64] = 256
    SW = 256
    PRW = 4 * SW  # 1024 per pair

    for q in range(NGRP):
        # ---- one-hots for 4 pairs: oh[e, (pr, c, j, v)] ----
        # ISA APs allow at most 3 free dims: merge (pair, chunk) -> a (16).
        # Pool may only TensorCopy/Memset on real hw (no PSUM, no
        # TensorTensor), so it materializes the edge broadcast (erep) and
        # DVE runs is_equal in 2x mode on the packed tile.
        erep = ohp.tile([128, GRP * PRW], BF16, tag="erep")
        erv = erep[:].rearrange("p (a j v) -> p a j v", a=4 * GRP, j=4)
        oh = ohp.tile([128, GRP * PRW], BF16, tag="oh")
        ohv = oh[:].rearrange("p (a j v) -> p a j v", a=4 * GRP, j=4)
        ev = etr[:, 16 * GRP * q:16 * GRP * (q + 1)].rearrange(
            "p (a j) -> p a j", j=4)
        e_all = ev.rearrange(
            "p a (j u) -> p a j u", u=1).to_broadcast([128, 4 * GRP, 4, 64])
        iota_s = iota[:].rearrange("p (a j v) -> p a j v",
                                   a=1, j=1).to_broadcast(
                                       [128, 4 * GRP, 2, 64])
        for jh in range(2):
            nc.gpsimd.tensor_copy(out=erv[:, :, 2 * jh:2 * jh + 2, :],
                                  in_=e_all[:, :, 2 * jh:2 * jh + 2, :])
            nc.vector.tensor_tensor(out=ohv[:, :, 2 * jh:2 * jh + 2, :],
                                    in0=erv[:, :, 2 * jh:2 * jh + 2, :],
                                    in1=iota_s, op=ISEQ)

        # ---- maug: per graph C[d, s] + I;  mst: transposed + scaled ----
        maug = gps.tile([128, 256], FP32, tag="maug")
        for pg in range(GRP):
            for gl in range(2):
                out_sl = maug[64 * gl:64 * gl + 64, 64 * pg:64 * pg + 64]
                tp = None if gl == 0 else (0, 64)
                for c in range(4):
                    base = PRW * pg + SW * c
                    lhsT = oh[:, base + 128 + 64 * gl:base + 192 + 64 * gl]
                    rhs = oh[:, base + 64 * gl:base + 64 * gl + 64]
                    nc.tensor.matmul(out_sl, lhsT, rhs, start=(c == 0),
                                     stop=False, tile_position=tp)
                nc.tensor.matmul(
                    out_sl, i64d[64 * gl:64 * gl + 64, :],
                    i64d[64 * gl:64 * gl + 64, :],
                    start=False, stop=True,
                    tile_position=None if gl == 0 else (64, 64),
                )

        mv = maug.rearrange("p (pr v) -> p pr v", pr=GRP)

        # ---- deg -> dinv = 1/sqrt(deg) ----
        deg = gsb.tile([128, GRP], FP32, tag="deg")
        nc.vector.tensor_reduce(out=deg[:], in_=mv,
                                axis=mybir.AxisListType.X,
                                op=mybir.AluOpType.add)
        sq = gsb.tile([128, GRP], FP32, tag="sq")
        nc.scalar.activation(out=sq[:], in_=deg[:], func=AF.Sqrt)
        dinv = gsb.tile([128, GRP], FP32, tag="dinv")
        nc.vector.reciprocal(out=dinv[:], in_=sq[:])
        dinv_b = dinv[:].rearrange("p (pr u) -> p pr u", u=1).to_broadcast(
            [128, GRP, 64])
        # (u=1 split keeps GRP in the first factor)

        # ---- msb = dinv_d * C, block-diag per pair (transpose outputs
        #      must land at PSUM partition 0, so transpose full 128x128) ----
        msb = gsb.tile([128, GRP * 128], FP32, tag="msb")
        if True:
            nc.gpsimd.memset(msb[:], 0)   # off-diag zeros, once per buffer
        for gl in range(2):
            h = slice(64 * gl, 64 * gl + 64)
            din_h = dinv[h, :].rearrange(
                "p (pr u) -> p pr u", u=1).to_broadcast([64, GRP, 64])
            nc.vector.tensor_tensor(
                out=msb[h, :].rearrange("p (pr v) -> p pr v",
                                        pr=GRP)[:, :, 64 * gl:64 * gl + 64],
                in0=maug[h, :].rearrange("p (pr v) -> p pr v", pr=GRP),
                in1=din_h, op=MUL)

        # ---- transpose per pair (fp32) -> mstb[s, d] block-diag ----
        mstb = gps.tile([128, GRP * 128], FP32, tag="mstb")
        for pg in range(GRP):
            sl = slice(128 * pg, 128 * pg + 128)
            nc.tensor.transpose(out=mstb[:, sl], in_=msb[:, sl],
                                identity=id1f[:])

        # ---- msbd = dinv_s * mstb  ( = As as [s, d] ) ----
        msbd = gsb.tile([128, GRP * 128], BF16, tag="msbd")
        dinv_b128 = dinv[:].rearrange(
            "p (pr u) -> p pr u", u=1).to_broadcast([128, GRP, 128])
        nc.vector.tensor_tensor(
            out=msbd[:].rearrange("p (pr v) -> p pr v", pr=GRP),
            in0=mstb[:].rearrange("p (pr v) -> p pr v", pr=GRP),
            in1=dinv_b128, op=MUL)

        # ---- tT[f, d] = sum_n x[n, f] As[d, n]  (pair-packed) ----
        tT = tps.tile([96, GRP * 128], FP32, tag="tT")
        for pg in range(GRP):
            pr = GRP * q + pg
            nc.tensor.matmul(
                tT[:, 128 * pg:128 * pg + 128],
                xt[:, 96 * pr:96 * pr + 96],
                msbd[:, 128 * pg:128 * pg + 128],
                start=True, stop=True,
            )
        tTs = gsb.tile([96, GRP * 128], BF16, tag="tTs")
        nc.scalar.activation(out=tTs[:], in_=tT[:], func=AF.Copy)

        # ---- per-pair: pre1T, relu, z2 ----
        z2 = tps.tile([128, GRP * 96], FP32, tag="z2")
        for pg in range(GRP):
            pre1 = pps.tile([128, 512], FP32, tag="pre1")
            for c in range(4):
                nc.tensor.matmul(pre1[:, 128 * c:128 * c + 128],
                                 w1t[:, 128 * c:128 * c + 128],
                                 tTs[:, 128 * pg:128 * pg + 128],
                                 start=True, stop=True)
            h1t = psb.tile([128, 512], BF16, tag="h1t")
            if has_b1:
                for c in range(4):
                    nc.scalar.activation(
                        out=h1t[:, 128 * c:128 * c + 128],
                        in_=pre1[:, 128 * c:128 * c + 128],
                        func=AF.Relu, bias=b1c[:, c:c + 1])
            elif pg % 2 == 0:
                nc.scalar.activation(out=h1t[:], in_=pre1[:], func=AF.Relu)
            else:
                nc.vector.tensor_scalar_max(h1t[:], pre1[:], 0.0)
            for c in range(4):
                nc.tensor.matmul(z2[:, 96 * pg:96 * pg + 96],
                                 h1t[:, 128 * c:128 * c + 128],
                                 w2t[:, 96 * c:96 * c + 96],
                                 start=(c == 0), stop=(c == 3))

        # (gpsimd cannot access PSUM on real hw - keep this on DVE)
        z2s = gsb.tile([128, GRP * 96], BF16, tag="z2s")
        nc.vector.tensor_copy(out=z2s[:], in_=z2[:])

        # ---- a2 = As z2 (per graph; block-diag lhsT slices) ----
        a2 = tps.tile([128, GRP * 96], FP32, tag="a2")
        for pg in range(GRP):
            for gl in range(2):
                h = slice(64 * gl, 64 * gl + 64)
                nc.tensor.matmul(
                    a2[h, 96 * pg:96 * pg + 96],
                    msbd[h, 128 * pg + 64 * gl:128 * pg + 64 * gl + 64],
                    z2s[h, 96 * pg:96 * pg + 96],
                    start=True, stop=True,
                    tile_position=None if gl == 0 else (64, 64),
                )

        # ---- hp = [h2 | h2] per pair (+ b2) ----
        hp = gsb.tile([128, GRP * 192], BF16, tag="hp")
        a2_b = a2[:].rearrange("p (pr t v) -> p pr t v", pr=GRP,
                               t=1).to_broadcast([128, GRP, 2, 96])
        hpv = hp[:].rearrange("p (pr t v) -> p pr t v", pr=GRP, t=2)
        nc.scalar.activation(out=hpv, in_=a2_b, func=AF.Copy)
        if has_b2:
            hpb = gsb.tile([128, GRP * 192], BF16, tag="hpb")
            nc.vector.tensor_tensor(
                out=hpb[:].rearrange("p (pr v) -> p pr v", pr=GRP),
                in0=hp[:].rearrange("p (pr v) -> p pr v", pr=GRP),
                in1=b2d[:].rearrange("p (t v) -> p t v", t=1).to_broadcast(
                    [128, GRP, 192]),
                op=mybir.AluOpType.add)
            hp = hpb

        # ---- conv per graph: out [o_lo, (oc, l)] ----
        for pg in range(GRP):
            pr = GRP * q + pg
            ysb = psb.tile([128, 768], BF16, tag="ysb")
            for gl in range(2):
                h = slice(64 * gl, 64 * gl + 64)
                yp = cps.tile([128, 384], FP32, tag="yp")
                for oc in range(4):
                    for k in range(3):
                        tap = (95, 0, 1)[k]
                        base = 192 * pg
                        nc.tensor.matmul(
                            yp[:, 96 * oc:96 * oc + 96],
                            cwd[h, 128 * (4 * k + oc):128 * (4 * k + oc) + 128],
                            hp[h, base + tap:base + tap + 96],
                            start=(k == 0), stop=(k == 2),
                        )
                if gl == 0:
                    nc.scalar.activation(out=ysb[:, 0:384], in_=yp[:],
                                         func=AF.Copy)
                else:
                    nc.vector.tensor_copy(out=ysb[:, 384:768], in_=yp[:])
            nc.sync.dma_start(out=y_d[pr], in_=ysb[:])

    ctx.close()


# ---------------- host side ----------------

def _prep_consts(W1, b1, W2, b2, conv_w):
    bf = ml_dtypes.bfloat16
    w1t = np.ascontiguousarray(W1.T).astype(bf)                    # [96, 512]
    w2t = np.ascontiguousarray(
        W2.T.reshape(4, 128, 96).transpose(1, 0, 2).reshape(128, 384)
    ).astype(bf)
    # cwd[i, (k, oc, o_lo)] = conv_w[oc*128+o_lo, i, k], duplicated rows
    base = np.ascontiguousarray(
        conv_w.transpose(1, 2, 0).reshape(64, 3 * 4 * 128))
    cwd = np.concatenate([base, base], axis=0).astype(bf)          # [128,1536]
    i64 = np.eye(64)
    i64d = np.concatenate([i64, i64], axis=0).astype(bf)           # [128, 64]
    iota = np.ascontiguousarray(
        np.broadcast_to(np.arange(64).astype(bf), (128, 64)))
    id1f = np.eye(128, dtype=np.float32)
    consts = dict(w1t=w1t, w2t=w2t, cwd=cwd, i64d=i64d, iota=iota, id1f=id1f)
    has_b1 = bool(np.any(b1))
    has_b2 = bool(np.any(b2))
    if has_b1:
        consts["b1c"] = np.ascontiguousarray(
            b1.reshape(4, 128).T).astype(np.float32)
    if has_b2:
        b2d = np.ascontiguousarray(
            np.broadcast_to(np.tile(b2, 2).astype(bf), (128, 192)))
        consts["b2d"] = b2d
    return consts, has_b1, has_b2


_NC_CACHE = {}


def _get_nc(has_b1, has_b2):
    key = (has_b1, has_b2)
    if key in _NC_CACHE:
        return _NC_CACHE[key]
    nc = bacc.Bacc("TRN2", target_bir_lowering=False, debug=False)
    ins = {
        "xt": nc.dram_tensor("xt", [128, 32 * 96], BF16,
                             kind="ExternalInput").ap(),
        "etr": nc.dram_tensor("etr", [128, 512], BF16,
                              kind="ExternalInput").ap(),
        "w1t": nc.dram_tensor("w1t", [96, 512], BF16,
                              kind="ExternalInput").ap(),
        "w2t": nc.dram_tensor("w2t", [128, 384], BF16,
                              kind="ExternalInput").ap(),
        "cwd": nc.dram_tensor("cwd", [128, 1536], BF16,
                              kind="ExternalInput").ap(),
        "i64d": nc.dram_tensor("i64d", [128, 64], BF16,
                               kind="ExternalInput").ap(),
        "iota": nc.dram_tensor("iota", [128, 64], BF16,
                               kind="ExternalInput").ap(),
        "id1f": nc.dram_tensor("id1f", [128, 128], FP32,
                               kind="ExternalInput").ap(),
    }
    if has_b1:
        ins["b1c"] = nc.dram_tensor("b1c", [128, 4], FP32,
                                    kind="ExternalInput").ap()
    if has_b2:
        ins["b2d"] = nc.dram_tensor("b2d", [128, 192], BF16,
                                    kind="ExternalInput").ap()
    outs = {
        "y": nc.dram_tensor("y", [NPAIR, 128, 768], BF16,
                            kind="ExternalOutput").ap(),
    }
    with tile.TileContext(nc) as tc:
        build_gcn_kernel(tc, outs, ins, has_b1, has_b2)
    nc.compile()
    _NC_CACHE[key] = nc
    return nc


def kernel(x, edge_index, W1, b1, W2, b2, conv_w, _trace=False):
    bf = ml_dtypes.bfloat16
    x = np.asarray(x)
    edge_index = np.asarray(edge_index)
    consts, has_b1, has_b2 = _prep_consts(
        np.asarray(W1), np.asarray(b1), np.asarray(W2), np.asarray(b2),
        np.asarray(conv_w))
    nc = _get_nc(has_b1, has_b2)

    in_maps = []
    for cid in range(N_CORES):
        sl = slice(cid * G, (cid + 1) * G)
        m = dict(consts)
        xc = np.asarray(x[sl])                       # [64, 96, 64]
        m["xt"] = np.ascontiguousarray(
            xc.reshape(32, 2, 96, 64).transpose(1, 3, 0, 2).reshape(
                128, 32 * 96)).astype(bf)
        ec = np.asarray(edge_index[sl])              # [64, 2, 512]
        m["etr"] = np.ascontiguousarray(
            ec.reshape(32, 2, 2, 4, 128).transpose(4, 0, 3, 2, 1).reshape(
                128, 512)).astype(bf)
        in_maps.append(m)

    res = run_bass_kernel_spmd(nc, in_maps, core_ids=list(range(N_CORES)),
                               trace=_trace)
    parts = []
    for cid in range(N_CORES):
        arr = np.asarray(res.results[cid]["y"])      # [32, 128, 768] bf16
        yc = arr.reshape(32, 128, 2, 4, 96).transpose(0, 2, 4, 3, 1)
        parts.append(yc.reshape(G, 96, 512).astype(np.float32))
    y = np.concatenate(parts, axis=0)
    if _trace:
        kernel.last_results = res
    return y


# revision 34
# speedup vs baseline: 1.3206x; 1.1528x over previous
"""Trainium2 Bass kernel for batched GCN (2x GCNConv + circular Conv1d).

Math per graph (N=64 nodes, S=96 feats, H=512 hidden, E=512 edges):
    deg[d]   = indegree + 1 (self loop)
    As       = Dinv (C + I) Dinv,  Dinv = diag(1/sqrt(deg)), C[d,s] counts
    h1       = relu((As X) W1^T + b1)          # aggregate-first (96-wide)
    h2       = As (h1 W2^T) + b2
    y        = circular_conv1d(h2, conv_w)     # emitted [o, l]-major

Device strategy (per core: 64 graphs = 32 pairs; pair nodes occupy
partition halves 0-63 / 64-127; pairs processed in groups of 4 to
amortize per-instruction init overheads on the elementwise engines):
  - edges host-transposed to [epos, (pair, chunk, j)] bf16; one-hots
    built by is_equal against an iota table with broadcast-input APs
    (src half on Pool/gpsimd, dst half on DVE) - no materialized
    broadcast copy.
  - C built per graph with K=128 one-hot matmuls + identity matmul
    (tile_position quadrants); deg via one batched reduce per group.
  - As assembled as [s, d]: row-scale by dinv_d, PE transpose (bf16 in,
    fp32 out, 1 cyc/row), row-scale by dinv_s; both GCN normalizations
    live in the matrix - no per-layer rescales.
  - layer1 aggregates x first (96-wide), then expands through W1 chunks
    transposed so layer2 needs no transposes.
  - conv as 12 96-col matmuls per graph ([o,l]-major output); output
    staged bf16, one DMA per pair; host undoes the layout + casts f32.
"""

import numpy as np
import ml_dtypes

import concourse.bacc as bacc
import concourse.mybir as mybir
import concourse.tile as tile
from concourse.bass_utils import run_bass_kernel_spmd

BF16 = mybir.dt.bfloat16
FP32 = mybir.dt.float32
AF = mybir.ActivationFunctionType
MUL = mybir.AluOpType.mult
ISEQ = mybir.AluOpType.is_equal

N_CORES = 8
B, S, N, H, E = 512, 96, 64, 512, 512
G = B // N_CORES          # graphs per core (64)
NPAIR = G // 2            # 32
GRP = 4                   # pairs per group
NGRP = NPAIR // GRP       # 8


def build_gcn_kernel(tc, outs, ins, has_b1=False, has_b2=False):
    nc = tc.nc

    xt_d = ins["xt"]        # [128, 32*96]  bf16  (r n) -> (q s)
    etr_d = ins["etr"]      # [128, 512]    bf16  epos -> (pr, c, j)
    w1t_d = ins["w1t"]      # [96, 512]     bf16  W1.T
    w2t_d = ins["w2t"]      # [128, 384]    bf16  (h_lo) -> (c, s)
    cwd_d = ins["cwd"]      # [128, 1536]   bf16  (dup i) -> (k, oc, o_lo)
    i64d_d = ins["i64d"]    # [128, 64]     bf16  I64 stacked twice
    iota_d = ins["iota"]    # [128, 64]     bf16  rows = 0..63
    y_d = outs["y"]         # [32, 128, 768] bf16

    from contextlib import ExitStack
    ctx = ExitStack()
    const = ctx.enter_context(tc.tile_pool(name="const", bufs=1))
    ohp = ctx.enter_context(tc.tile_pool(name="ohp", bufs=3))
    gsb = ctx.enter_context(tc.tile_pool(name="gsb", bufs=3))
    psb = ctx.enter_context(tc.tile_pool(name="psb", bufs=4))
    # PSUM is bank-granular (8 x 2KB). Tiles with disjoint lifetimes share
    # a bank so every tag can double-buffer: big = maug -> mstb -> tT,
    # za = z2 -> a2.
    gps = ctx.enter_context(tc.tile_pool(name="gps", bufs=2, space="PSUM"))
    tps = ctx.enter_context(tc.tile_pool(name="tps", bufs=2, space="PSUM"))
    pps = ctx.enter_context(tc.tile_pool(name="pps", bufs=2, space="PSUM"))
    cps = ctx.enter_context(tc.tile_pool(name="cps", bufs=2, space="PSUM"))

    # ---- constants ----
    xt = const.tile([128, 32 * 96], BF16)
    nc.sync.dma_start(out=xt[:], in_=xt_d[:])
    etr = const.tile([128, 512], BF16)
    nc.sync.dma_start(out=etr[:], in_=etr_d[:])
    w1t = const.tile([96, 512], BF16)
    nc.sync.dma_start(out=w1t[:], in_=w1t_d[:])
    w2t = const.tile([128, 384], BF16)
    nc.sync.dma_start(out=w2t[:], in_=w2t_d[:])
    cwd = const.tile([128, 1536], BF16)
    nc.sync.dma_start(out=cwd[:], in_=cwd_d[:])
    i64d = const.tile([128, 64], BF16)
    nc.sync.dma_start(out=i64d[:], in_=i64d_d[:])
    iota = const.tile([128, 64], BF16)
    nc.sync.dma_start(out=iota[:], in_=iota_d[:])
    id1f = const.tile([128, 128], FP32)
    nc.sync.dma_start(out=id1f[:], in_=ins["id1f"][:])
    if has_b1:
        b1c = const.tile([128, 4], FP32)
        nc.sync.dma_start(out=b1c[:], in_=ins["b1c"][:])
    if has_b2:
        b2d = const.tile([128, 192], BF16)
        nc.sync.dma_start(out=b2d[:], in_=ins["b2d"][:])

    # per-(pair,chunk) col layout inside oh:
    #   [src0 64 | src1 64 | dst0 64 | dst1 64] = 256
    SW = 256
    PRW = 4 * SW  # 1024 per pair

    for q in range(NGRP):
        # ---- one-hots for 4 pairs: oh[e, (pr, c, j, v)] ----
        # ISA APs allow at most 3 free dims: merge (pair, chunk) -> a (16).
        # Pool may only TensorCopy/Memset on real hw (no PSUM, no
        # TensorTensor), so it materializes the edge broadcast (erep) and
        # DVE runs is_equal in 2x mode on the packed tile.
        erep = ohp.tile([128, GRP * PRW], BF16, tag="erep")
        erv = erep[:].rearrange("p (a j v) -> p a j v", a=4 * GRP, j=4)
        oh = ohp.tile([128, GRP * PRW], BF16, tag="oh")
        ohv = oh[:].rearrange("p (a j v) -> p a j v", a=4 * GRP, j=4)
        ev = etr[:, 16 * GRP * q:16 * GRP * (q + 1)].rearrange(
            "p (a j) -> p a j", j=4)
        e_all = ev.rearrange(
            "p a (j u) -> p a j u", u=1).to_broadcast([128, 4 * GRP, 4, 64])
        iota_s = iota[:].rearrange("p (a j v) -> p a j v",
                                   a=1, j=1).to_broadcast(
                                       [128, 4 * GRP, 2, 64])
        for jh in range(2):
            nc.gpsimd.tensor_copy(out=erv[:, :, 2 * jh:2 * jh + 2, :],
                                  in_=e_all[:, :, 2 * jh:2 * jh + 2, :])
            nc.vector.tensor_tensor(out=ohv[:, :, 2 * jh:2 * jh + 2, :],
                                    in0=erv[:, :, 2 * jh:2 * jh + 2, :],
                                    in1=iota_s, op=ISEQ)

        # ---- maug: per graph C[d, s] + I ----
        big = gps.tile([128, 512], FP32, tag="big")
        maug = big[:, 0:256]
        for pg in range(GRP):
            for gl in range(2):
                out_sl = maug[64 * gl:64 * gl + 64, 64 * pg:64 * pg + 64]
                tp = None if gl == 0 else (0, 64)
                for c in range(4):
                    base = PRW * pg + SW * c
                    lhsT = oh[:, base + 128 + 64 * gl:base + 192 + 64 * gl]
                    rhs = oh[:, base + 64 * gl:base + 64 * gl + 64]
                    nc.tensor.matmul(out_sl, lhsT, rhs, start=(c == 0),
                                     stop=False, tile_position=tp)
                nc.tensor.matmul(
                    out_sl, i64d[64 * gl:64 * gl + 64, :],
                    i64d[64 * gl:64 * gl + 64, :],
                    start=False, stop=True,
                    tile_position=None if gl == 0 else (64, 64),
                )

        mv = maug.rearrange("p (pr v) -> p pr v", pr=GRP)

        # ---- deg -> dinv = 1/sqrt(deg) ----
        deg = gsb.tile([128, GRP], FP32, tag="deg")
        nc.vector.tensor_reduce(out=deg[:], in_=mv,
                                axis=mybir.AxisListType.X,
                                op=mybir.AluOpType.add)
        sq = gsb.tile([128, GRP], FP32, tag="sq")
        nc.scalar.activation(out=sq[:], in_=deg[:], func=AF.Sqrt)
        dinv = gsb.tile([128, GRP], FP32, tag="dinv")
        nc.vector.reciprocal(out=dinv[:], in_=sq[:])
        dinv_b = dinv[:].rearrange("p (pr u) -> p pr u", u=1).to_broadcast(
            [128, GRP, 64])
        # (u=1 split keeps GRP in the first factor)

        # ---- msb = dinv_d * C, block-diag per pair (transpose outputs
        #      must land at PSUM partition 0, so transpose full 128x128) ----
        msb = gsb.tile([128, GRP * 128], FP32, tag="msb")
        if True:
            nc.gpsimd.memset(msb[:], 0)   # off-diag zeros, once per buffer
        for gl in range(2):
            h = slice(64 * gl, 64 * gl + 64)
            din_h = dinv[h, :].rearrange(
                "p (pr u) -> p pr u", u=1).to_broadcast([64, GRP, 64])
            nc.vector.tensor_tensor(
                out=msb[h, :].rearrange("p (pr v) -> p pr v",
                                        pr=GRP)[:, :, 64 * gl:64 * gl + 64],
                in0=maug[h, :].rearrange("p (pr v) -> p pr v", pr=GRP),
                in1=din_h, op=MUL)

        # ---- transpose per pair (fp32) -> mstb[s, d] block-diag;
        #      reuses the maug bank (maug is dead after the scales) ----
        mstb = big
        for pg in range(GRP):
            sl = slice(128 * pg, 128 * pg + 128)
            nc.tensor.transpose(out=mstb[:, sl], in_=msb[:, sl],
                                identity=id1f[:])

        # ---- msbd = dinv_s * mstb  ( = As as [s, d] ) ----
        msbd = gsb.tile([128, GRP * 128], BF16, tag="msbd")
        dinv_b128 = dinv[:].rearrange(
            "p (pr u) -> p pr u", u=1).to_broadcast([128, GRP, 128])
        nc.vector.tensor_tensor(
            out=msbd[:].rearrange("p (pr v) -> p pr v", pr=GRP),
            in0=mstb[:].rearrange("p (pr v) -> p pr v", pr=GRP),
            in1=dinv_b128, op=MUL)

        # ---- tT[f, d] = sum_n x[n, f] As[d, n]  (pair-packed);
        #      reuses the same bank again (mstb dead after msbd) ----
        tT = big[0:96, :]
        for pg in range(GRP):
            pr = GRP * q + pg
            nc.tensor.matmul(
                tT[:, 128 * pg:128 * pg + 128],
                xt[:, 96 * pr:96 * pr + 96],
                msbd[:, 128 * pg:128 * pg + 128],
                start=True, stop=True,
            )
        tTs = gsb.tile([96, GRP * 128], BF16, tag="tTs")
        nc.scalar.activation(out=tTs[:], in_=tT[:], func=AF.Copy)

        # ---- per-pair: pre1T, relu, z2 ----
        za = tps.tile([128, GRP * 96], FP32, tag="za")
        z2 = za
        for pg in range(GRP):
            pre1 = pps.tile([128, 512], FP32, tag="pre1")
            for c in range(4):
                nc.tensor.matmul(pre1[:, 128 * c:128 * c + 128],
                                 w1t[:, 128 * c:128 * c + 128],
                                 tTs[:, 128 * pg:128 * pg + 128],
                                 start=True, stop=True)
            h1t = psb.tile([128, 512], BF16, tag="h1t")
            if has_b1:
                for c in range(4):
                    nc.scalar.activation(
                        out=h1t[:, 128 * c:128 * c + 128],
                        in_=pre1[:, 128 * c:128 * c + 128],
                        func=AF.Relu, bias=b1c[:, c:c + 1])
            else:
                nc.scalar.activation(out=h1t[:], in_=pre1[:], func=AF.Relu)
            for c in range(4):
                nc.tensor.matmul(z2[:, 96 * pg:96 * pg + 96],
                                 h1t[:, 128 * c:128 * c + 128],
                                 w2t[:, 96 * c:96 * c + 96],
                                 start=(c == 0), stop=(c == 3))

        # (gpsimd cannot access PSUM on real hw - keep off Pool)
        z2s = gsb.tile([128, GRP * 96], BF16, tag="z2s")
        nc.scalar.activation(out=z2s[:], in_=z2[:], func=AF.Copy)

        # ---- a2 = As z2 (per graph; block-diag lhsT slices);
        #      reuses the z2 bank (z2 dead after z2s) ----
        a2 = za
        for pg in range(GRP):
            for gl in range(2):
                h = slice(64 * gl, 64 * gl + 64)
                nc.tensor.matmul(
                    a2[h, 96 * pg:96 * pg + 96],
                    msbd[h, 128 * pg + 64 * gl:128 * pg + 64 * gl + 64],
                    z2s[h, 96 * pg:96 * pg + 96],
                    start=True, stop=True,
                    tile_position=None if gl == 0 else (64, 64),
                )

        # ---- hp = [h2 | h2] per pair (+ b2) ----
        hp = gsb.tile([128, GRP * 192], BF16, tag="hp")
        a2_b = a2[:].rearrange("p (pr t v) -> p pr t v", pr=GRP,
                               t=1).to_broadcast([128, GRP, 2, 96])
        hpv = hp[:].rearrange("p (pr t v) -> p pr t v", pr=GRP, t=2)
        nc.scalar.activation(out=hpv, in_=a2_b, func=AF.Copy)
        if has_b2:
            hpb = gsb.tile([128, GRP * 192], BF16, tag="hpb")
            nc.vector.tensor_tensor(
                out=hpb[:].rearrange("p (pr v) -> p pr v", pr=GRP),
                in0=hp[:].rearrange("p (pr v) -> p pr v", pr=GRP),
                in1=b2d[:].rearrange("p (t v) -> p t v", t=1).to_broadcast(
                    [128, GRP, 192]),
                op=mybir.AluOpType.add)
            hp = hpb

        # ---- conv per graph: out [o_lo, (oc, l)] ----
        for pg in range(GRP):
            pr = GRP * q + pg
            ysb = psb.tile([128, 768], BF16, tag="ysb")
            for gl in range(2):
                h = slice(64 * gl, 64 * gl + 64)
                yp = cps.tile([128, 384], FP32, tag="yp")
                for oc in range(4):
                    for k in range(3):
                        tap = (95, 0, 1)[k]
                        base = 192 * pg
                        nc.tensor.matmul(
                            yp[:, 96 * oc:96 * oc + 96],
                            cwd[h, 128 * (4 * k + oc):128 * (4 * k + oc) + 128],
                            hp[h, base + tap:base + tap + 96],
                            start=(k == 0), stop=(k == 2),
                        )
                if gl == 0:
                    nc.scalar.activation(out=ysb[:, 0:384], in_=yp[:],
                                         func=AF.Copy)
                else:
                    nc.vector.tensor_copy(out=ysb[:, 384:768], in_=yp[:])
            nc.sync.dma_start(out=y_d[pr], in_=ysb[:])

    ctx.close()


# ---------------- host side ----------------

def _prep_consts(W1, b1, W2, b2, conv_w):
    bf = ml_dtypes.bfloat16
    w1t = np.ascontiguousarray(W1.T).astype(bf)                    # [96, 512]
    w2t = np.ascontiguousarray(
        W2.T.reshape(4, 128, 96).transpose(1, 0, 2).reshape(128, 384)
    ).astype(bf)
    # cwd[i, (k, oc, o_lo)] = conv_w[oc*128+o_lo, i, k], duplicated rows
    base = np.ascontiguousarray(
        conv_w.transpose(1, 2, 0).reshape(64, 3 * 4 * 128))
    cwd = np.concatenate([base, base], axis=0).astype(bf)          # [128,1536]
    i64 = np.eye(64)
    i64d = np.concatenate([i64, i64], axis=0).astype(bf)           # [128, 64]
    iota = np.ascontiguousarray(
        np.broadcast_to(np.arange(64).astype(bf), (128, 64)))
    id1f = np.eye(128, dtype=np.float32)
    consts = dict(w1t=w1t, w2t=w2t, cwd=cwd, i64d=i64d, iota=iota, id1f=id1f)
    has_b1 = bool(np.any(b1))
    has_b2 = bool(np.any(b2))
    if has_b1:
        consts["b1c"] = np.ascontiguousarray(
            b1.reshape(4, 128).T).astype(np.float32)
    if has_b2:
        b2d = np.ascontiguousarray(
            np.broadcast_to(np.tile(b2, 2).astype(bf), (128, 192)))
        consts["b2d"] = b2d
    return consts, has_b1, has_b2


_NC_CACHE = {}


def _get_nc(has_b1, has_b2):
    key = (has_b1, has_b2)
    if key in _NC_CACHE:
        return _NC_CACHE[key]
    nc = bacc.Bacc("TRN2", target_bir_lowering=False, debug=False)
    ins = {
        "xt": nc.dram_tensor("xt", [128, 32 * 96], BF16,
                             kind="ExternalInput").ap(),
        "etr": nc.dram_tensor("etr", [128, 512], BF16,
                              kind="ExternalInput").ap(),
        "w1t": nc.dram_tensor("w1t", [96, 512], BF16,
                              kind="ExternalInput").ap(),
        "w2t": nc.dram_tensor("w2t", [128, 384], BF16,
                              kind="ExternalInput").ap(),
        "cwd": nc.dram_tensor("cwd", [128, 1536], BF16,
                              kind="ExternalInput").ap(),
        "i64d": nc.dram_tensor("i64d", [128, 64], BF16,
                               kind="ExternalInput").ap(),
        "iota": nc.dram_tensor("iota", [128, 64], BF16,
                               kind="ExternalInput").ap(),
        "id1f": nc.dram_tensor("id1f", [128, 128], FP32,
                               kind="ExternalInput").ap(),
    }
    if has_b1:
        ins["b1c"] = nc.dram_tensor("b1c", [128, 4], FP32,
                                    kind="ExternalInput").ap()
    if has_b2:
        ins["b2d"] = nc.dram_tensor("b2d", [128, 192], BF16,
                                    kind="ExternalInput").ap()
    outs = {
        "y": nc.dram_tensor("y", [NPAIR, 128, 768], BF16,
                            kind="ExternalOutput").ap(),
    }
    with tile.TileContext(nc) as tc:
        build_gcn_kernel(tc, outs, ins, has_b1, has_b2)
    nc.compile()
    _NC_CACHE[key] = nc
    return nc


def kernel(x, edge_index, W1, b1, W2, b2, conv_w, _trace=False):
    bf = ml_dtypes.bfloat16
    x = np.asarray(x)
    edge_index = np.asarray(edge_index)
    consts, has_b1, has_b2 = _prep_consts(
        np.asarray(W1), np.asarray(b1), np.asarray(W2), np.asarray(b2),
        np.asarray(conv_w))
    nc = _get_nc(has_b1, has_b2)

    in_maps = []
    for cid in range(N_CORES):
        sl = slice(cid * G, (cid + 1) * G)
        m = dict(consts)
        xc = np.asarray(x[sl])                       # [64, 96, 64]
        m["xt"] = np.ascontiguousarray(
            xc.reshape(32, 2, 96, 64).transpose(1, 3, 0, 2).reshape(
                128, 32 * 96)).astype(bf)
        ec = np.asarray(edge_index[sl])              # [64, 2, 512]
        m["etr"] = np.ascontiguousarray(
            ec.reshape(32, 2, 2, 4, 128).transpose(4, 0, 3, 2, 1).reshape(
                128, 512)).astype(bf)
        in_maps.append(m)

    res = run_bass_kernel_spmd(nc, in_maps, core_ids=list(range(N_CORES)),
                               trace=_trace)
    parts = []
    for cid in range(N_CORES):
        arr = np.asarray(res.results[cid]["y"])      # [32, 128, 768] bf16
        yc = arr.reshape(32, 128, 2, 4, 96).transpose(0, 2, 4, 3, 1)
        parts.append(yc.reshape(G, 96, 512).astype(np.float32))
    y = np.concatenate(parts, axis=0)
    if _trace:
        kernel.last_results = res
    return y


# revision 41
# speedup vs baseline: 1.3809x; 1.0457x over previous
"""Trainium2 Bass kernel for batched GCN (2x GCNConv + circular Conv1d).

Math per graph (N=64 nodes, S=96 feats, H=512 hidden, E=512 edges):
    deg[d]   = indegree + 1 (self loop)
    As       = Dinv (C + I) Dinv,  Dinv = diag(1/sqrt(deg)), C[d,s] counts
    h1       = relu((As X) W1^T + b1)          # aggregate-first (96-wide)
    h2       = As (h1 W2^T) + b2
    y        = circular_conv1d(h2, conv_w)     # emitted [o, l]-major

Device strategy (per core: 64 graphs = 32 pairs; pair nodes occupy
partition halves 0-63 / 64-127; pairs processed in groups of 4 to
amortize per-instruction init overheads on the elementwise engines):
  - edges host-transposed to [epos, (pair, chunk, j)] bf16; one-hots
    built by is_equal against an iota table with broadcast-input APs
    (src half on Pool/gpsimd, dst half on DVE) - no materialized
    broadcast copy.
  - C built per graph with K=128 one-hot matmuls + identity matmul
    (tile_position quadrants); deg via one batched reduce per group.
  - As assembled as [s, d]: row-scale by dinv_d, PE transpose (bf16 in,
    fp32 out, 1 cyc/row), row-scale by dinv_s; both GCN normalizations
    live in the matrix - no per-layer rescales.
  - layer1 aggregates x first (96-wide), then expands through W1 chunks
    transposed so layer2 needs no transposes.
  - conv as 12 96-col matmuls per graph ([o,l]-major output); output
    staged bf16, one DMA per pair; host undoes the layout + casts f32.
"""

import numpy as np
import ml_dtypes

import concourse.bacc as bacc
import concourse.mybir as mybir
import concourse.tile as tile
from concourse.bass_utils import run_bass_kernel_spmd

BF16 = mybir.dt.bfloat16
FP32 = mybir.dt.float32
AF = mybir.ActivationFunctionType
MUL = mybir.AluOpType.mult
ISEQ = mybir.AluOpType.is_equal

N_CORES = 8
B, S, N, H, E = 512, 96, 64, 512, 512
G = B // N_CORES          # graphs per core (64)
NPAIR = G // 2            # 32
GRP = 4                   # pairs per group
NGRP = NPAIR // GRP       # 8


def build_gcn_kernel(tc, outs, ins, has_b1=False, has_b2=False):
    nc = tc.nc

    xt_d = ins["xt"]        # [128, 32*96]  bf16  (r n) -> (q s)
    etr_d = ins["etr"]      # [128, 512]    bf16  epos -> (pr, c, j)
    w1t_d = ins["w1t"]      # [96, 512]     bf16  W1.T
    w2t_d = ins["w2t"]      # [128, 384]    bf16  (h_lo) -> (c, s)
    cwd_d = ins["cwd"]      # [128, 1536]   bf16  (dup i) -> (k, oc, o_lo)
    i64d_d = ins["i64d"]    # [128, 64]     bf16  I64 stacked twice
    iota_d = ins["iota"]    # [128, 64]     bf16  rows = 0..63
    y_d = outs["y"]         # [32, 128, 768] bf16

    from contextlib import ExitStack
    ctx = ExitStack()
    const = ctx.enter_context(tc.tile_pool(name="const", bufs=1))
    ohp = ctx.enter_context(tc.tile_pool(name="ohp", bufs=3))
    gsb = ctx.enter_context(tc.tile_pool(name="gsb", bufs=3))
    psb = ctx.enter_context(tc.tile_pool(name="psb", bufs=4))
    # PSUM is bank-granular (8 x 2KB). Tiles with disjoint lifetimes share
    # a bank so every tag can double-buffer: big = maug -> mstb -> tT,
    # za = z2 -> a2.
    gps = ctx.enter_context(tc.tile_pool(name="gps", bufs=2, space="PSUM"))
    tps = ctx.enter_context(tc.tile_pool(name="tps", bufs=2, space="PSUM"))
    pps = ctx.enter_context(tc.tile_pool(name="pps", bufs=2, space="PSUM"))
    cps = ctx.enter_context(tc.tile_pool(name="cps", bufs=2, space="PSUM"))

    # ---- constants (edge/iota first: they gate the pipeline head) ----
    etr = const.tile([128, 512], BF16)
    nc.sync.dma_start(out=etr[:], in_=etr_d[:])
    iota = const.tile([128, 64], BF16)
    nc.sync.dma_start(out=iota[:], in_=iota_d[:])
    i64d = const.tile([128, 64], BF16)
    nc.sync.dma_start(out=i64d[:], in_=i64d_d[:])
    id1f = const.tile([128, 128], FP32)
    nc.sync.dma_start(out=id1f[:], in_=ins["id1f"][:])
    xt = const.tile([128, 32 * 96], BF16)
    nc.sync.dma_start(out=xt[:], in_=xt_d[:])
    w1t = const.tile([96, 512], BF16)
    nc.sync.dma_start(out=w1t[:], in_=w1t_d[:])
    w2t = const.tile([128, 384], BF16)
    nc.sync.dma_start(out=w2t[:], in_=w2t_d[:])
    cwd = const.tile([128, 1536], BF16)
    nc.sync.dma_start(out=cwd[:], in_=cwd_d[:])
    if has_b1:
        b1c = const.tile([128, 4], FP32)
        nc.sync.dma_start(out=b1c[:], in_=ins["b1c"][:])
    if has_b2:
        b2d = const.tile([128, 192], BF16)
        nc.sync.dma_start(out=b2d[:], in_=ins["b2d"][:])

    # per-(pair,chunk) col layout inside oh:
    #   [src0 64 | src1 64 | dst0 64 | dst1 64] = 256
    SW = 256
    PRW = 4 * SW  # 1024 per pair

    for q in range(NGRP):
        # ---- one-hots for 4 pairs: oh[e, (pr, c, j, v)] ----
        # ISA APs allow at most 3 free dims: merge (pair, chunk) -> a (16).
        # Pool may only TensorCopy/Memset on real hw (no PSUM, no
        # TensorTensor), so it materializes the edge broadcast (erep) and
        # DVE runs is_equal in 2x mode on the packed tile.
        erep = ohp.tile([128, GRP * PRW], BF16, tag="erep")
        erv = erep[:].rearrange("p (a j v) -> p a j v", a=4 * GRP, j=4)
        oh = ohp.tile([128, GRP * PRW], BF16, tag="oh")
        ohv = oh[:].rearrange("p (a j v) -> p a j v", a=4 * GRP, j=4)
        ev = etr[:, 16 * GRP * q:16 * GRP * (q + 1)].rearrange(
            "p (a j) -> p a j", j=4)
        e_all = ev.rearrange(
            "p a (j u) -> p a j u", u=1).to_broadcast([128, 4 * GRP, 4, 64])
        iota_s = iota[:].rearrange("p (a j v) -> p a j v",
                                   a=1, j=1).to_broadcast(
                                       [128, 4 * GRP, 2, 64])
        iota_s1 = iota[:].rearrange("p (a j v) -> p a j v",
                                    a=1, j=1).to_broadcast(
                                        [128, 4 * GRP, 1, 64])
        for jh in range(4):
            nc.gpsimd.tensor_copy(out=erv[:, :, jh:jh + 1, :],
                                  in_=e_all[:, :, jh:jh + 1, :])
            nc.vector.tensor_tensor(out=ohv[:, :, jh:jh + 1, :],
                                    in0=erv[:, :, jh:jh + 1, :],
                                    in1=iota_s1, op=ISEQ)

        # ---- maug: per graph C[d, s] + I ----
        big = gps.tile([128, 512], FP32, tag="big")
        maug = big[:, 0:256]
        for pg in range(GRP):
            for gl in range(2):
                out_sl = maug[64 * gl:64 * gl + 64, 64 * pg:64 * pg + 64]
                tp = None if gl == 0 else (0, 64)
                for c in range(4):
                    base = PRW * pg + SW * c
                    lhsT = oh[:, base + 128 + 64 * gl:base + 192 + 64 * gl]
                    rhs = oh[:, base + 64 * gl:base + 64 * gl + 64]
                    nc.tensor.matmul(out_sl, lhsT, rhs, start=(c == 0),
                                     stop=False, tile_position=tp)
                nc.tensor.matmul(
                    out_sl, i64d[64 * gl:64 * gl + 64, :],
                    i64d[64 * gl:64 * gl + 64, :],
                    start=False, stop=True,
                    tile_position=None if gl == 0 else (64, 64),
                )

        mv = maug.rearrange("p (pr v) -> p pr v", pr=GRP)

        # ---- deg -> dinv = 1/sqrt(deg) ----
        deg = gsb.tile([128, GRP], FP32, tag="deg")
        nc.vector.tensor_reduce(out=deg[:], in_=mv,
                                axis=mybir.AxisListType.X,
                                op=mybir.AluOpType.add)
        sq = gsb.tile([128, GRP], FP32, tag="sq")
        nc.scalar.activation(out=sq[:], in_=deg[:], func=AF.Sqrt)
        dinv = gsb.tile([128, GRP], FP32, tag="dinv")
        nc.vector.reciprocal(out=dinv[:], in_=sq[:])
        dinv_b = dinv[:].rearrange("p (pr u) -> p pr u", u=1).to_broadcast(
            [128, GRP, 64])
        # (u=1 split keeps GRP in the first factor)

        # ---- msb = dinv_d * C, block-diag per pair (transpose outputs
        #      must land at PSUM partition 0, so transpose full 128x128) ----
        msb = gsb.tile([128, GRP * 128], FP32, tag="msb")
        if True:
            nc.gpsimd.memset(msb[:], 0)   # off-diag zeros, once per buffer
        for gl in range(2):
            h = slice(64 * gl, 64 * gl + 64)
            din_h = dinv[h, :].rearrange(
                "p (pr u) -> p pr u", u=1).to_broadcast([64, GRP, 64])
            nc.vector.tensor_tensor(
                out=msb[h, :].rearrange("p (pr v) -> p pr v",
                                        pr=GRP)[:, :, 64 * gl:64 * gl + 64],
                in0=maug[h, :].rearrange("p (pr v) -> p pr v", pr=GRP),
                in1=din_h, op=MUL)

        # ---- transpose per pair (fp32) -> mstb[s, d] block-diag;
        #      reuses the maug bank (maug is dead after the scales) ----
        mstb = big
        for pg in range(GRP):
            sl = slice(128 * pg, 128 * pg + 128)
            nc.tensor.transpose(out=mstb[:, sl], in_=msb[:, sl],
                                identity=id1f[:])

        # ---- msbd = dinv_s * mstb  ( = As as [s, d] ) ----
        msbd = gsb.tile([128, GRP * 128], BF16, tag="msbd")
        dinv_b128 = dinv[:].rearrange(
            "p (pr u) -> p pr u", u=1).to_broadcast([128, GRP, 128])
        nc.vector.tensor_tensor(
            out=msbd[:].rearrange("p (pr v) -> p pr v", pr=GRP),
            in0=mstb[:].rearrange("p (pr v) -> p pr v", pr=GRP),
            in1=dinv_b128, op=MUL)

        # ---- tT[f, d] = sum_n x[n, f] As[d, n]  (pair-packed);
        #      reuses the same bank again (mstb dead after msbd) ----
        tT = big[0:96, :]
        for pg in range(GRP):
            pr = GRP * q + pg
            nc.tensor.matmul(
                tT[:, 128 * pg:128 * pg + 128],
                xt[:, 96 * pr:96 * pr + 96],
                msbd[:, 128 * pg:128 * pg + 128],
                start=True, stop=True,
            )
        tTs = gsb.tile([96, GRP * 128], BF16, tag="tTs")
        nc.scalar.activation(out=tTs[:], in_=tT[:], func=AF.Copy)

        # ---- per-pair: pre1T, relu, z2 ----
        za = tps.tile([128, GRP * 96], FP32, tag="za")
        z2 = za
        for pg in range(GRP):
            pre1 = pps.tile([128, 512], FP32, tag="pre1")
            for c in range(4):
                nc.tensor.matmul(pre1[:, 128 * c:128 * c + 128],
                                 w1t[:, 128 * c:128 * c + 128],
                                 tTs[:, 128 * pg:128 * pg + 128],
                                 start=True, stop=True)
            h1t = psb.tile([128, 512], BF16, tag="h1t")
            if has_b1:
                for c in range(4):
                    nc.scalar.activation(
                        out=h1t[:, 128 * c:128 * c + 128],
                        in_=pre1[:, 128 * c:128 * c + 128],
                        func=AF.Relu, bias=b1c[:, c:c + 1])
            else:
                nc.scalar.activation(out=h1t[:], in_=pre1[:], func=AF.Relu)
            for c in range(4):
                nc.tensor.matmul(z2[:, 96 * pg:96 * pg + 96],
                                 h1t[:, 128 * c:128 * c + 128],
                                 w2t[:, 96 * c:96 * c + 96],
                                 start=(c == 0), stop=(c == 3))

        # (gpsimd cannot access PSUM on real hw - keep off Pool)
        z2s = gsb.tile([128, GRP * 96], BF16, tag="z2s")
        nc.scalar.activation(out=z2s[:], in_=z2[:], func=AF.Copy)

        # ---- a2 = As z2 (per graph; block-diag lhsT slices);
        #      reuses the z2 bank (z2 dead after z2s) ----
        a2 = za
        for pg in range(GRP):
            for gl in range(2):
                h = slice(64 * gl, 64 * gl + 64)
                nc.tensor.matmul(
                    a2[h, 96 * pg:96 * pg + 96],
                    msbd[h, 128 * pg + 64 * gl:128 * pg + 64 * gl + 64],
                    z2s[h, 96 * pg:96 * pg + 96],
                    start=True, stop=True,
                    tile_position=None if gl == 0 else (64, 64),
                )

        # ---- hp = [h2 | h2] per pair (+ b2) ----
        hp = gsb.tile([128, GRP * 192], BF16, tag="hp")
        a2_b = a2[:].rearrange("p (pr t v) -> p pr t v", pr=GRP,
                               t=1).to_broadcast([128, GRP, 2, 96])
        hpv = hp[:].rearrange("p (pr t v) -> p pr t v", pr=GRP, t=2)
        nc.scalar.activation(out=hpv, in_=a2_b, func=AF.Copy)
        if has_b2:
            hpb = gsb.tile([128, GRP * 192], BF16, tag="hpb")
            nc.vector.tensor_tensor(
                out=hpb[:].rearrange("p (pr v) -> p pr v", pr=GRP),
                in0=hp[:].rearrange("p (pr v) -> p pr v", pr=GRP),
                in1=b2d[:].rearrange("p (t v) -> p t v", t=1).to_broadcast(
                    [128, GRP, 192]),
                op=mybir.AluOpType.add)
            hp = hpb

        # ---- conv: units of (gl, oc), each 3 tap-matmuls spanning all 4
        #      pairs (384 cols); out [o_lo, (pr, l)] ----
        ysb = psb.tile([128, GRP * 768], BF16, tag="ysb")
        ysv = ysb[:].rearrange("p (pr r) -> p pr r", pr=GRP)
        for gl in range(2):
            h = slice(64 * gl, 64 * gl + 64)
            for oc in range(4):
                yp = cps.tile([128, 384], FP32, tag="yp")
                for k in range(3):
                    tap = (95, 0, 1)[k]
                    nc.tensor.matmul(
                        yp[:].rearrange("p (pr v) -> p pr v", pr=GRP),
                        cwd[h, 128 * (4 * k + oc):128 * (4 * k + oc) + 128],
                        hp[64 * gl:64 * gl + 64, :].rearrange(
                            "p (pr v) -> p pr v", pr=GRP)[:, :, tap:tap + 96],
                        start=(k == 0), stop=(k == 2),
                    )
                co = 384 * gl + 96 * oc
                if (gl + oc) % 2 == 0:
                    nc.scalar.activation(
                        out=ysv[:, :, co:co + 96],
                        in_=yp[:].rearrange("p (pr v) -> p pr v", pr=GRP),
                        func=AF.Copy)
                else:
                    nc.vector.tensor_copy(
                        out=ysv[:, :, co:co + 96],
                        in_=yp[:].rearrange("p (pr v) -> p pr v", pr=GRP))
        nc.sync.dma_start(out=y_d[q], in_=ysb[:])

    ctx.close()


# ---------------- host side ----------------

def _prep_consts(W1, b1, W2, b2, conv_w):
    bf = ml_dtypes.bfloat16
    w1t = np.ascontiguousarray(W1.T).astype(bf)                    # [96, 512]
    w2t = np.ascontiguousarray(
        W2.T.reshape(4, 128, 96).transpose(1, 0, 2).reshape(128, 384)
    ).astype(bf)
    # cwd[i, (k, oc, o_lo)] = conv_w[oc*128+o_lo, i, k], duplicated rows
    base = np.ascontiguousarray(
        conv_w.transpose(1, 2, 0).reshape(64, 3 * 4 * 128))
    cwd = np.concatenate([base, base], axis=0).astype(bf)          # [128,1536]
    i64 = np.eye(64)
    i64d = np.concatenate([i64, i64], axis=0).astype(bf)           # [128, 64]
    iota = np.ascontiguousarray(
        np.broadcast_to(np.arange(64).astype(bf), (128, 64)))
    id1f = np.eye(128, dtype=np.float32)
    consts = dict(w1t=w1t, w2t=w2t, cwd=cwd, i64d=i64d, iota=iota, id1f=id1f)
    has_b1 = bool(np.any(b1))
    has_b2 = bool(np.any(b2))
    if has_b1:
        consts["b1c"] = np.ascontiguousarray(
            b1.reshape(4, 128).T).astype(np.float32)
    if has_b2:
        b2d = np.ascontiguousarray(
            np.broadcast_to(np.tile(b2, 2).astype(bf), (128, 192)))
        consts["b2d"] = b2d
    return consts, has_b1, has_b2


_NC_CACHE = {}


def _get_nc(has_b1, has_b2):
    key = (has_b1, has_b2)
    if key in _NC_CACHE:
        return _NC_CACHE[key]
    nc = bacc.Bacc("TRN2", target_bir_lowering=False, debug=False)
    ins = {
        "xt": nc.dram_tensor("xt", [128, 32 * 96], BF16,
                             kind="ExternalInput").ap(),
        "etr": nc.dram_tensor("etr", [128, 512], BF16,
                              kind="ExternalInput").ap(),
        "w1t": nc.dram_tensor("w1t", [96, 512], BF16,
                              kind="ExternalInput").ap(),
        "w2t": nc.dram_tensor("w2t", [128, 384], BF16,
                              kind="ExternalInput").ap(),
        "cwd": nc.dram_tensor("cwd", [128, 1536], BF16,
                              kind="ExternalInput").ap(),
        "i64d": nc.dram_tensor("i64d", [128, 64], BF16,
                               kind="ExternalInput").ap(),
        "iota": nc.dram_tensor("iota", [128, 64], BF16,
                               kind="ExternalInput").ap(),
        "id1f": nc.dram_tensor("id1f", [128, 128], FP32,
                               kind="ExternalInput").ap(),
    }
    if has_b1:
        ins["b1c"] = nc.dram_tensor("b1c", [128, 4], FP32,
                                    kind="ExternalInput").ap()
    if has_b2:
        ins["b2d"] = nc.dram_tensor("b2d", [128, 192], BF16,
                                    kind="ExternalInput").ap()
    outs = {
        "y": nc.dram_tensor("y", [NGRP, 128, GRP * 768], BF16,
                            kind="ExternalOutput").ap(),
    }
    with tile.TileContext(nc) as tc:
        build_gcn_kernel(tc, outs, ins, has_b1, has_b2)
    nc.compile()
    _NC_CACHE[key] = nc
    return nc


def kernel(x, edge_index, W1, b1, W2, b2, conv_w, _trace=False):
    bf = ml_dtypes.bfloat16
    x = np.asarray(x)
    edge_index = np.asarray(edge_index)
    consts, has_b1, has_b2 = _prep_consts(
        np.asarray(W1), np.asarray(b1), np.asarray(W2), np.asarray(b2),
        np.asarray(conv_w))
    nc = _get_nc(has_b1, has_b2)

    in_maps = []
    for cid in range(N_CORES):
        sl = slice(cid * G, (cid + 1) * G)
        m = dict(consts)
        xc = np.asarray(x[sl])                       # [64, 96, 64]
        m["xt"] = np.ascontiguousarray(
            xc.reshape(32, 2, 96, 64).transpose(1, 3, 0, 2).reshape(
                128, 32 * 96)).astype(bf)
        ec = np.asarray(edge_index[sl])              # [64, 2, 512]
        m["etr"] = np.ascontiguousarray(
            ec.reshape(32, 2, 2, 4, 128).transpose(4, 0, 3, 2, 1).reshape(
                128, 512)).astype(bf)
        in_maps.append(m)

    res = run_bass_kernel_spmd(nc, in_maps, core_ids=list(range(N_CORES)),
                               trace=_trace)
    parts = []
    for cid in range(N_CORES):
        arr = np.asarray(res.results[cid]["y"])      # [8, 128, 3072] bf16
        yc = arr.reshape(NGRP, 128, GRP, 2, 4, 96).transpose(0, 2, 3, 5, 4, 1)
        parts.append(yc.reshape(G, 96, 512).astype(np.float32))
    y = np.concatenate(parts, axis=0)
    if _trace:
        kernel.last_results = res
    return y


# revision 42
# speedup vs baseline: 1.4339x; 1.0384x over previous
"""Trainium2 Bass kernel for batched GCN (2x GCNConv + circular Conv1d).

Math per graph (N=64 nodes, S=96 feats, H=512 hidden, E=512 edges):
    deg[d]   = indegree + 1 (self loop)
    As       = Dinv (C + I) Dinv,  Dinv = diag(1/sqrt(deg)), C[d,s] counts
    h1       = relu((As X) W1^T + b1)          # aggregate-first (96-wide)
    h2       = As (h1 W2^T) + b2
    y        = circular_conv1d(h2, conv_w)     # emitted [o, l]-major

Device strategy (per core: 64 graphs = 32 pairs; pair nodes occupy
partition halves 0-63 / 64-127; pairs processed in groups - tapered
2,2,4..4,2,2 so pipeline fill/drain is short - to amortize per-op init
overheads on the elementwise engines):
  - edges host-transposed to [epos, (pair, chunk, j)] bf16; Pool
    materializes the edge broadcast (it may only TensorCopy/Memset on
    real hw), DVE runs is_equal in 2x mode against an iota table.
  - C built per graph with K=128 one-hot matmuls + identity matmul
    (tile_position quadrants); deg via one batched reduce per group.
  - As assembled block-diag [s, d]: row-scale dinv_d, one 128x128 PE
    transpose per pair (transpose outs must start at PSUM partition 0),
    row-scale dinv_s; both GCN normalizations live in the matrix.
  - layer1 aggregates x first (96-wide), then expands through W1 chunks
    transposed so layer2 needs no transposes.
  - conv as (gl,oc) units: 3 tap-matmuls spanning the whole group;
    output staged bf16, one DMA per group; host undoes layout + casts.
  - PSUM is bank-granular (8 x 2KB): tiles with disjoint lifetimes
    share banks (big = maug->mstb->tT, za = z2->a2) so every tag
    double-buffers and groups pipeline.
"""

import numpy as np
import ml_dtypes

import concourse.bacc as bacc
import concourse.mybir as mybir
import concourse.tile as tile
from concourse.bass_utils import run_bass_kernel_spmd

BF16 = mybir.dt.bfloat16
FP32 = mybir.dt.float32
AF = mybir.ActivationFunctionType
MUL = mybir.AluOpType.mult
ISEQ = mybir.AluOpType.is_equal

N_CORES = 8
B, S, N, H, E = 512, 96, 64, 512, 512
G = B // N_CORES          # graphs per core (64)
NPAIR = G // 2            # 32
GRP = 4                   # max pairs per group (tile sizing)
GROUPS = [2, 2, 4, 4, 4, 4, 4, 4, 2, 2]
assert sum(GROUPS) == NPAIR

SW = 256                  # oh cols per (pair, chunk)
PRW = 4 * SW              # oh cols per pair


def _emit_group(nc, P, q0, gs, has_b1, has_b2):
    """Emit one group of gs pairs starting at pair q0."""
    (const, ohp, gsb, psb, gps, tps, pps, cps, xt, etr, w1t, w2t, cwd, i64d,
     iota, id1f, b1c, b2d, y_d) = P

    # ---- one-hots: oh[e, (pr, c, j, v)]  (a = pr*4+c merged) ----
    na = 4 * gs
    erep = ohp.tile([128, GRP * PRW], BF16, tag="erep")
    erv = erep[:, 0:gs * PRW].rearrange("p (a j v) -> p a j v", a=na, j=4)
    oh = ohp.tile([128, GRP * PRW], BF16, tag="oh")
    ohv = oh[:, 0:gs * PRW].rearrange("p (a j v) -> p a j v", a=na, j=4)
    ev = etr[:, 16 * q0:16 * (q0 + gs)].rearrange("p (a j) -> p a j", j=4)
    e_all = ev.rearrange(
        "p a (j u) -> p a j u", u=1).to_broadcast([128, na, 4, 64])
    iota_s1 = iota[:].rearrange(
        "p (a j v) -> p a j v", a=1, j=1).to_broadcast([128, na, 1, 64])
    for jh in range(4):
        nc.gpsimd.tensor_copy(out=erv[:, :, jh:jh + 1, :],
                              in_=e_all[:, :, jh:jh + 1, :])
        nc.vector.tensor_tensor(out=ohv[:, :, jh:jh + 1, :],
                                in0=erv[:, :, jh:jh + 1, :],
                                in1=iota_s1, op=ISEQ)

    # ---- maug: per graph C[d, s] + I ----
    big = gps.tile([128, 512], FP32, tag="big")
    maug = big[:, 0:64 * gs]
    for pg in range(gs):
        for gl in range(2):
            out_sl = maug[64 * gl:64 * gl + 64, 64 * pg:64 * pg + 64]
            tp = None if gl == 0 else (0, 64)
            for c in range(4):
                base = PRW * pg + SW * c
                lhsT = oh[:, base + 128 + 64 * gl:base + 192 + 64 * gl]
                rhs = oh[:, base + 64 * gl:base + 64 * gl + 64]
                nc.tensor.matmul(out_sl, lhsT, rhs, start=(c == 0),
                                 stop=False, tile_position=tp)
            nc.tensor.matmul(
                out_sl, i64d[64 * gl:64 * gl + 64, :],
                i64d[64 * gl:64 * gl + 64, :],
                start=False, stop=True,
                tile_position=None if gl == 0 else (64, 64),
            )

    mv = maug.rearrange("p (pr v) -> p pr v", pr=gs)

    # ---- deg -> dinv = 1/sqrt(deg) ----
    deg_t = gsb.tile([128, GRP], FP32, tag="deg")
    deg = deg_t[:, 0:gs]
    nc.vector.tensor_reduce(out=deg, in_=mv, axis=mybir.AxisListType.X,
                            op=mybir.AluOpType.add)
    sq_t = gsb.tile([128, GRP], FP32, tag="sq")
    sq = sq_t[:, 0:gs]
    nc.scalar.activation(out=sq, in_=deg, func=AF.Sqrt)
    dinv_t = gsb.tile([128, GRP], FP32, tag="dinv")
    dinv = dinv_t[:, 0:gs]
    nc.vector.reciprocal(out=dinv, in_=sq)

    # ---- msb = dinv_d * C, block-diag per pair ----
    msb = gsb.tile([128, GRP * 128], FP32, tag="msb")
    nc.gpsimd.memset(msb[:, 0:128 * gs], 0)
    for gl in range(2):
        h = slice(64 * gl, 64 * gl + 64)
        din_h = dinv[h, :].rearrange(
            "p (pr u) -> p pr u", u=1).to_broadcast([64, gs, 64])
        nc.vector.tensor_tensor(
            out=msb[h, 0:128 * gs].rearrange(
                "p (pr v) -> p pr v", pr=gs)[:, :, 64 * gl:64 * gl + 64],
            in0=maug[h, :].rearrange("p (pr v) -> p pr v", pr=gs),
            in1=din_h, op=MUL)

    # ---- transpose per pair (fp32) -> mstb block-diag (reuses maug bank)
    mstb = big[:, 0:128 * gs]
    for pg in range(gs):
        sl = slice(128 * pg, 128 * pg + 128)
        nc.tensor.transpose(out=mstb[:, sl], in_=msb[:, sl],
                            identity=id1f[:])

    # ---- msbd = dinv_s * mstb  ( = As as [s, d] ) ----
    msbd_t = gsb.tile([128, GRP * 128], BF16, tag="msbd")
    msbd = msbd_t[:, 0:128 * gs]
    dinv_b128 = dinv[:, :].rearrange(
        "p (pr u) -> p pr u", u=1).to_broadcast([128, gs, 128])
    nc.vector.tensor_tensor(
        out=msbd.rearrange("p (pr v) -> p pr v", pr=gs),
        in0=mstb.rearrange("p (pr v) -> p pr v", pr=gs),
        in1=dinv_b128, op=MUL)

    # ---- tT[f, d] (pair-packed; reuses the same bank after msbd) ----
    tT = big[0:96, 0:128 * gs]
    for pg in range(gs):
        nc.tensor.matmul(
            tT[:, 128 * pg:128 * pg + 128],
            xt[:, 96 * (q0 + pg):96 * (q0 + pg) + 96],
            msbd[:, 128 * pg:128 * pg + 128],
            start=True, stop=True,
        )
    tTs_t = gsb.tile([96, GRP * 128], BF16, tag="tTs")
    tTs = tTs_t[:, 0:128 * gs]
    nc.scalar.activation(out=tTs, in_=tT, func=AF.Copy)

    # ---- per-pair: pre1T, relu, z2 ----
    za = tps.tile([128, GRP * 96], FP32, tag="za")
    z2 = za[:, 0:96 * gs]
    for pg in range(gs):
        pre1 = pps.tile([128, 512], FP32, tag="pre1")
        for c in range(4):
            nc.tensor.matmul(pre1[:, 128 * c:128 * c + 128],
                             w1t[:, 128 * c:128 * c + 128],
                             tTs[:, 128 * pg:128 * pg + 128],
                             start=True, stop=True)
        h1t = psb.tile([128, 512], BF16, tag="h1t")
        if has_b1:
            for c in range(4):
                nc.scalar.activation(
                    out=h1t[:, 128 * c:128 * c + 128],
                    in_=pre1[:, 128 * c:128 * c + 128],
                    func=AF.Relu, bias=b1c[:, c:c + 1])
        else:
            nc.scalar.activation(out=h1t[:], in_=pre1[:], func=AF.Relu)
        for c in range(4):
            nc.tensor.matmul(z2[:, 96 * pg:96 * pg + 96],
                             h1t[:, 128 * c:128 * c + 128],
                             w2t[:, 96 * c:96 * c + 96],
                             start=(c == 0), stop=(c == 3))

    # (gpsimd cannot access PSUM on real hw - keep off Pool)
    z2s_t = gsb.tile([128, GRP * 96], BF16, tag="z2s")
    z2s = z2s_t[:, 0:96 * gs]
    nc.scalar.activation(out=z2s, in_=z2, func=AF.Copy)

    # ---- a2 = As z2 (per graph; reuses the z2 bank) ----
    a2 = za[:, 0:96 * gs]
    for pg in range(gs):
        for gl in range(2):
            h = slice(64 * gl, 64 * gl + 64)
            nc.tensor.matmul(
                a2[h, 96 * pg:96 * pg + 96],
                msbd[h, 128 * pg + 64 * gl:128 * pg + 64 * gl + 64],
                z2s[h, 96 * pg:96 * pg + 96],
                start=True, stop=True,
                tile_position=None if gl == 0 else (64, 64),
            )

    # ---- hp = [h2 | h2] per pair (+ b2) ----
    hp_t = gsb.tile([128, GRP * 192], BF16, tag="hp")
    hp = hp_t[:, 0:192 * gs]
    a2_b = a2.rearrange("p (pr t v) -> p pr t v", pr=gs,
                        t=1).to_broadcast([128, gs, 2, 96])
    hpv = hp.rearrange("p (pr t v) -> p pr t v", pr=gs, t=2)
    nc.scalar.activation(out=hpv, in_=a2_b, func=AF.Copy)
    if has_b2:
        hpb_t = gsb.tile([128, GRP * 192], BF16, tag="hpb")
        hpb = hpb_t[:, 0:192 * gs]
        nc.vector.tensor_tensor(
            out=hpb.rearrange("p (pr v) -> p pr v", pr=gs),
            in0=hp.rearrange("p (pr v) -> p pr v", pr=gs),
            in1=b2d[:].rearrange("p (t v) -> p t v", t=1).to_broadcast(
                [128, gs, 192]),
            op=mybir.AluOpType.add)
        hp = hpb

    # ---- conv: units of (gl, oc), 3 tap-matmuls spanning the group ----
    ysb = psb.tile([128, GRP * 768], BF16, tag="ysb")
    ysv = ysb[:, 0:768 * gs].rearrange("p (pr r) -> p pr r", pr=gs)
    for gl in range(2):
        h = slice(64 * gl, 64 * gl + 64)
        for oc in range(4):
            yp = cps.tile([128, 384], FP32, tag="yp")
            ypv = yp[:, 0:96 * gs].rearrange("p (pr v) -> p pr v", pr=gs)
            for k in range(3):
                tap = (95, 0, 1)[k]
                nc.tensor.matmul(
                    ypv,
                    cwd[h, 128 * (4 * k + oc):128 * (4 * k + oc) + 128],
                    hp[64 * gl:64 * gl + 64, :].rearrange(
                        "p (pr v) -> p pr v", pr=gs)[:, :, tap:tap + 96],
                    start=(k == 0), stop=(k == 2),
                )
            co = 384 * gl + 96 * oc
            if (gl + oc) % 2 == 0:
                nc.scalar.activation(out=ysv[:, :, co:co + 96], in_=ypv,
                                     func=AF.Copy)
            else:
                nc.vector.tensor_copy(out=ysv[:, :, co:co + 96], in_=ypv)
    nc.sync.dma_start(
        out=y_d[q0:q0 + gs].rearrange("g p v -> p g v"),
        in_=ysb[:, 0:768 * gs].rearrange("p (g v) -> p g v", g=gs))


def build_gcn_kernel(tc, outs, ins, has_b1=False, has_b2=False):
    nc = tc.nc
    y_d = outs["y"]         # [32, 128, 768] bf16

    from contextlib import ExitStack
    ctx = ExitStack()
    const = ctx.enter_context(tc.tile_pool(name="const", bufs=1))
    ohp = ctx.enter_context(tc.tile_pool(name="ohp", bufs=3))
    gsb = ctx.enter_context(tc.tile_pool(name="gsb", bufs=3))
    psb = ctx.enter_context(tc.tile_pool(name="psb", bufs=4))
    gps = ctx.enter_context(tc.tile_pool(name="gps", bufs=2, space="PSUM"))
    tps = ctx.enter_context(tc.tile_pool(name="tps", bufs=2, space="PSUM"))
    pps = ctx.enter_context(tc.tile_pool(name="pps", bufs=2, space="PSUM"))
    cps = ctx.enter_context(tc.tile_pool(name="cps", bufs=2, space="PSUM"))

    # ---- constants (edge/iota first: they gate the pipeline head) ----
    etr = const.tile([128, 512], BF16)
    nc.sync.dma_start(out=etr[:], in_=ins["etr"][:])
    iota = const.tile([128, 64], BF16)
    nc.sync.dma_start(out=iota[:], in_=ins["iota"][:])
    i64d = const.tile([128, 64], BF16)
    nc.sync.dma_start(out=i64d[:], in_=ins["i64d"][:])
    id1f = const.tile([128, 128], FP32)
    nc.sync.dma_start(out=id1f[:], in_=ins["id1f"][:])
    xt = const.tile([128, 32 * 96], BF16)
    nc.sync.dma_start(out=xt[:], in_=ins["xt"][:])
    w1t = const.tile([96, 512], BF16)
    nc.sync.dma_start(out=w1t[:], in_=ins["w1t"][:])
    w2t = const.tile([128, 384], BF16)
    nc.sync.dma_start(out=w2t[:], in_=ins["w2t"][:])
    cwd = const.tile([128, 1536], BF16)
    nc.sync.dma_start(out=cwd[:], in_=ins["cwd"][:])
    b1c = b2d = None
    if has_b1:
        b1c = const.tile([128, 4], FP32)
        nc.sync.dma_start(out=b1c[:], in_=ins["b1c"][:])
    if has_b2:
        b2d = const.tile([128, 192], BF16)
        nc.sync.dma_start(out=b2d[:], in_=ins["b2d"][:])

    P = (const, ohp, gsb, psb, gps, tps, pps, cps, xt, etr, w1t, w2t, cwd,
         i64d, iota, id1f, b1c, b2d, y_d)
    q0 = 0
    for gs in GROUPS:
        _emit_group(nc, P, q0, gs, has_b1, has_b2)
        q0 += gs

    ctx.close()


# ---------------- host side ----------------

def _prep_consts(W1, b1, W2, b2, conv_w):
    bf = ml_dtypes.bfloat16
    w1t = np.ascontiguousarray(W1.T).astype(bf)                    # [96, 512]
    w2t = np.ascontiguousarray(
        W2.T.reshape(4, 128, 96).transpose(1, 0, 2).reshape(128, 384)
    ).astype(bf)
    # cwd[i, (k, oc, o_lo)] = conv_w[oc*128+o_lo, i, k], duplicated rows
    base = np.ascontiguousarray(
        conv_w.transpose(1, 2, 0).reshape(64, 3 * 4 * 128))
    cwd = np.concatenate([base, base], axis=0).astype(bf)          # [128,1536]
    i64 = np.eye(64)
    i64d = np.concatenate([i64, i64], axis=0).astype(bf)           # [128, 64]
    iota = np.ascontiguousarray(
        np.broadcast_to(np.arange(64).astype(bf), (128, 64)))
    id1f = np.eye(128, dtype=np.float32)
    consts = dict(w1t=w1t, w2t=w2t, cwd=cwd, i64d=i64d, iota=iota, id1f=id1f)
    has_b1 = bool(np.any(b1))
    has_b2 = bool(np.any(b2))
    if has_b1:
        consts["b1c"] = np.ascontiguousarray(
            b1.reshape(4, 128).T).astype(np.float32)
    if has_b2:
        b2d = np.ascontiguousarray(
            np.broadcast_to(np.tile(b2, 2).astype(bf), (128, 192)))
        consts["b2d"] = b2d
    return consts, has_b1, has_b2


_NC_CACHE = {}


def _get_nc(has_b1, has_b2):
    key = (has_b1, has_b2)
    if key in _NC_CACHE:
        return _NC_CACHE[key]
    nc = bacc.Bacc("TRN2", target_bir_lowering=False, debug=False)
    ins = {
        "xt": nc.dram_tensor("xt", [128, 32 * 96], BF16,
                             kind="ExternalInput").ap(),
        "etr": nc.dram_tensor("etr", [128, 512], BF16,
                              kind="ExternalInput").ap(),
        "w1t": nc.dram_tensor("w1t", [96, 512], BF16,
                              kind="ExternalInput").ap(),
        "w2t": nc.dram_tensor("w2t", [128, 384], BF16,
                              kind="ExternalInput").ap(),
        "cwd": nc.dram_tensor("cwd", [128, 1536], BF16,
                              kind="ExternalInput").ap(),
        "i64d": nc.dram_tensor("i64d", [128, 64], BF16,
                               kind="ExternalInput").ap(),
        "iota": nc.dram_tensor("iota", [128, 64], BF16,
                               kind="ExternalInput").ap(),
        "id1f": nc.dram_tensor("id1f", [128, 128], FP32,
                               kind="ExternalInput").ap(),
    }
    if has_b1:
        ins["b1c"] = nc.dram_tensor("b1c", [128, 4], FP32,
                                    kind="ExternalInput").ap()
    if has_b2:
        ins["b2d"] = nc.dram_tensor("b2d", [128, 192], BF16,
                                    kind="ExternalInput").ap()
    outs = {
        "y": nc.dram_tensor("y", [NPAIR, 128, 768], BF16,
                            kind="ExternalOutput").ap(),
    }
    with tile.TileContext(nc) as tc:
        build_gcn_kernel(tc, outs, ins, has_b1, has_b2)
    nc.compile()
    _NC_CACHE[key] = nc
    return nc


def kernel(x, edge_index, W1, b1, W2, b2, conv_w, _trace=False):
    bf = ml_dtypes.bfloat16
    x = np.asarray(x)
    edge_index = np.asarray(edge_index)
    consts, has_b1, has_b2 = _prep_consts(
        np.asarray(W1), np.asarray(b1), np.asarray(W2), np.asarray(b2),
        np.asarray(conv_w))
    nc = _get_nc(has_b1, has_b2)

    in_maps = []
    for cid in range(N_CORES):
        sl = slice(cid * G, (cid + 1) * G)
        m = dict(consts)
        xc = np.asarray(x[sl])                       # [64, 96, 64]
        m["xt"] = np.ascontiguousarray(
            xc.reshape(32, 2, 96, 64).transpose(1, 3, 0, 2).reshape(
                128, 32 * 96)).astype(bf)
        ec = np.asarray(edge_index[sl])              # [64, 2, 512]
        m["etr"] = np.ascontiguousarray(
            ec.reshape(32, 2, 2, 4, 128).transpose(4, 0, 3, 2, 1).reshape(
                128, 512)).astype(bf)
        in_maps.append(m)

    res = run_bass_kernel_spmd(nc, in_maps, core_ids=list(range(N_CORES)),
                               trace=_trace)
    parts = []
    for cid in range(N_CORES):
        arr = np.asarray(res.results[cid]["y"])      # [32, 128, 768] bf16
        yc = arr.reshape(32, 128, 2, 4, 96).transpose(0, 2, 4, 3, 1)
        parts.append(yc.reshape(G, 96, 512).astype(np.float32))
    y = np.concatenate(parts, axis=0)
    if _trace:
        kernel.last_results = res
    return y


# revision 44
# speedup vs baseline: 1.6505x; 1.1511x over previous
"""Trainium2 Bass kernel for batched GCN (2x GCNConv + circular Conv1d).

Math per graph (N=64 nodes, S=96 feats, H=512 hidden, E=512 edges):
    deg[d]   = indegree + 1 (self loop)
    As       = Dinv (C + I) Dinv,  Dinv = diag(1/sqrt(deg)), C[d,s] counts
    h1       = relu((As X) W1^T + b1)          # aggregate-first (96-wide)
    h2       = As (h1 W2^T) + b2
    y        = circular_conv1d(h2, conv_w)     # emitted [o, l]-major

Device strategy (per core: 64 graphs = 32 pairs; pair nodes occupy
partition halves 0-63 / 64-127; pairs processed in groups - tapered
2,2,4..4,2,2 so pipeline fill/drain is short - to amortize per-op init
overheads on the elementwise engines):
  - edges host-transposed to [epos, (pair, chunk, j)] bf16; Pool
    materializes the edge broadcast (it may only TensorCopy/Memset on
    real hw), DVE runs is_equal in 2x mode against an iota table.
  - C built per graph with K=128 one-hot matmuls + identity matmul
    (tile_position quadrants); deg via one batched reduce per group.
  - As assembled block-diag [s, d]: row-scale dinv_d, one 128x128 PE
    transpose per pair (transpose outs must start at PSUM partition 0),
    row-scale dinv_s; both GCN normalizations live in the matrix.
  - layer1 aggregates x first (96-wide), then expands through W1 chunks
    transposed so layer2 needs no transposes.
  - conv as (gl,oc) units: 3 tap-matmuls spanning the whole group;
    output staged bf16, one DMA per group; host undoes layout + casts.
  - PSUM is bank-granular (8 x 2KB): tiles with disjoint lifetimes
    share banks (big = maug->mstb->tT, za = z2->a2) so every tag
    double-buffers and groups pipeline.
"""

import numpy as np
import ml_dtypes

import concourse.bacc as bacc
import concourse.mybir as mybir
import concourse.tile as tile
from concourse.bass_utils import run_bass_kernel_spmd

BF16 = mybir.dt.bfloat16
FP32 = mybir.dt.float32
AF = mybir.ActivationFunctionType
MUL = mybir.AluOpType.mult
ISEQ = mybir.AluOpType.is_equal

N_CORES = 8
B, S, N, H, E = 512, 96, 64, 512, 512
G = B // N_CORES          # graphs per core (64)
NPAIR = G // 2            # 32
GRP = 4                   # max pairs per group (tile sizing)
GROUPS = [2, 2, 4, 4, 4, 4, 4, 4, 2, 2]
assert sum(GROUPS) == NPAIR

SW = 256                  # oh cols per (pair, chunk)
PRW = 4 * SW              # oh cols per pair


def _emit_group(nc, P, q0, gs, has_b1, has_b2):
    """Emit one group of gs pairs starting at pair q0."""
    (const, ohp, gsb, psb, gps, tps, pps, cps, xt, etr, w1t, w2t, cwd, i64d,
     iota, id1f, b1c, b2d, y_d) = P

    # ---- one-hots: oh[e, (pr, c, j, v)]  (a = pr*4+c merged) ----
    # Pool (the only engine free for it) replicates each edge id just 16x;
    # DVE compares the packed replica against 4 shifted iota slices in 2x
    # mode. This keeps Pool off the critical path (it was saturated when
    # it broadcast the full 64).
    na = 4 * gs
    erep = ohp.tile([128, GRP * 256], BF16, tag="erep")
    erv = erep[:, 0:gs * 256].rearrange("p (a j r) -> p a j r", a=na, j=4)
    oh = ohp.tile([128, GRP * PRW], BF16, tag="oh")
    ohv = oh[:, 0:gs * PRW].rearrange("p (a j v) -> p a j v", a=na, j=4)
    ev = etr[:, 16 * q0:16 * (q0 + gs)].rearrange("p (a j) -> p a j", j=4)
    e_all = ev.rearrange(
        "p a (j u) -> p a j u", u=1).to_broadcast([128, na, 4, 16])
    for jh in range(4):
        nc.gpsimd.tensor_copy(out=erv[:, :, jh:jh + 1, :],
                              in_=e_all[:, :, jh:jh + 1, :])
        for qv in range(4):
            iota_r = iota[:, 16 * qv:16 * qv + 16].rearrange(
                "p (a j r) -> p a j r", a=1, j=1).to_broadcast(
                    [128, na, 1, 16])
            nc.vector.tensor_tensor(
                out=ohv[:, :, jh:jh + 1, 16 * qv:16 * qv + 16],
                in0=erv[:, :, jh:jh + 1, :],
                in1=iota_r, op=ISEQ)

    # ---- maug: per graph C[d, s] + I ----
    big = gps.tile([128, 512], FP32, tag="big")
    maug = big[:, 0:64 * gs]
    for pg in range(gs):
        for gl in range(2):
            out_sl = maug[64 * gl:64 * gl + 64, 64 * pg:64 * pg + 64]
            tp = None if gl == 0 else (0, 64)
            for c in range(4):
                base = PRW * pg + SW * c
                lhsT = oh[:, base + 128 + 64 * gl:base + 192 + 64 * gl]
                rhs = oh[:, base + 64 * gl:base + 64 * gl + 64]
                nc.tensor.matmul(out_sl, lhsT, rhs, start=(c == 0),
                                 stop=False, tile_position=tp)
            nc.tensor.matmul(
                out_sl, i64d[64 * gl:64 * gl + 64, :],
                i64d[64 * gl:64 * gl + 64, :],
                start=False, stop=True,
                tile_position=None if gl == 0 else (64, 64),
            )

    mv = maug.rearrange("p (pr v) -> p pr v", pr=gs)

    # ---- deg -> dinv = 1/sqrt(deg) ----
    deg_t = gsb.tile([128, GRP], FP32, tag="deg")
    deg = deg_t[:, 0:gs]
    nc.vector.tensor_reduce(out=deg, in_=mv, axis=mybir.AxisListType.X,
                            op=mybir.AluOpType.add)
    sq_t = gsb.tile([128, GRP], FP32, tag="sq")
    sq = sq_t[:, 0:gs]
    nc.scalar.activation(out=sq, in_=deg, func=AF.Sqrt)
    dinv_t = gsb.tile([128, GRP], FP32, tag="dinv")
    dinv = dinv_t[:, 0:gs]
    nc.vector.reciprocal(out=dinv, in_=sq)

    # ---- msb = dinv_d * C, block-diag per pair ----
    msb = gsb.tile([128, GRP * 128], FP32, tag="msb")
    nc.gpsimd.memset(msb[:, 0:128 * gs], 0)
    for gl in range(2):
        h = slice(64 * gl, 64 * gl + 64)
        din_h = dinv[h, :].rearrange(
            "p (pr u) -> p pr u", u=1).to_broadcast([64, gs, 64])
        nc.vector.tensor_tensor(
            out=msb[h, 0:128 * gs].rearrange(
                "p (pr v) -> p pr v", pr=gs)[:, :, 64 * gl:64 * gl + 64],
            in0=maug[h, :].rearrange("p (pr v) -> p pr v", pr=gs),
            in1=din_h, op=MUL)

    # ---- transpose per pair (fp32) -> mstb block-diag (reuses maug bank)
    mstb = big[:, 0:128 * gs]
    for pg in range(gs):
        sl = slice(128 * pg, 128 * pg + 128)
        nc.tensor.transpose(out=mstb[:, sl], in_=msb[:, sl],
                            identity=id1f[:])

    # ---- msbd = dinv_s * mstb  ( = As as [s, d] ) ----
    msbd_t = gsb.tile([128, GRP * 128], BF16, tag="msbd")
    msbd = msbd_t[:, 0:128 * gs]
    dinv_b128 = dinv[:, :].rearrange(
        "p (pr u) -> p pr u", u=1).to_broadcast([128, gs, 128])
    nc.vector.tensor_tensor(
        out=msbd.rearrange("p (pr v) -> p pr v", pr=gs),
        in0=mstb.rearrange("p (pr v) -> p pr v", pr=gs),
        in1=dinv_b128, op=MUL)

    # ---- tT[f, d] (pair-packed; reuses the same bank after msbd) ----
    tT = big[0:96, 0:128 * gs]
    for pg in range(gs):
        nc.tensor.matmul(
            tT[:, 128 * pg:128 * pg + 128],
            xt[:, 96 * (q0 + pg):96 * (q0 + pg) + 96],
            msbd[:, 128 * pg:128 * pg + 128],
            start=True, stop=True,
        )
    tTs_t = gsb.tile([96, GRP * 128], BF16, tag="tTs")
    tTs = tTs_t[:, 0:128 * gs]
    nc.scalar.activation(out=tTs, in_=tT, func=AF.Copy)

    # ---- per-pair: pre1T, relu, z2 ----
    za = tps.tile([128, GRP * 96], FP32, tag="za")
    z2 = za[:, 0:96 * gs]
    for pg in range(gs):
        pre1 = pps.tile([128, 512], FP32, tag="pre1")
        for c in range(4):
            nc.tensor.matmul(pre1[:, 128 * c:128 * c + 128],
                             w1t[:, 128 * c:128 * c + 128],
                             tTs[:, 128 * pg:128 * pg + 128],
                             start=True, stop=True)
        h1t = psb.tile([128, 512], BF16, tag="h1t")
        if has_b1:
            for c in range(4):
                nc.scalar.activation(
                    out=h1t[:, 128 * c:128 * c + 128],
                    in_=pre1[:, 128 * c:128 * c + 128],
                    func=AF.Relu, bias=b1c[:, c:c + 1])
        else:
            nc.scalar.activation(out=h1t[:], in_=pre1[:], func=AF.Relu)
        for c in range(4):
            nc.tensor.matmul(z2[:, 96 * pg:96 * pg + 96],
                             h1t[:, 128 * c:128 * c + 128],
                             w2t[:, 96 * c:96 * c + 96],
                             start=(c == 0), stop=(c == 3))

    # (gpsimd cannot access PSUM on real hw - keep off Pool)
    z2s_t = gsb.tile([128, GRP * 96], BF16, tag="z2s")
    z2s = z2s_t[:, 0:96 * gs]
    nc.scalar.activation(out=z2s, in_=z2, func=AF.Copy)

    # ---- a2 = As z2 (per graph; reuses the z2 bank) ----
    a2 = za[:, 0:96 * gs]
    for pg in range(gs):
        for gl in range(2):
            h = slice(64 * gl, 64 * gl + 64)
            nc.tensor.matmul(
                a2[h, 96 * pg:96 * pg + 96],
                msbd[h, 128 * pg + 64 * gl:128 * pg + 64 * gl + 64],
                z2s[h, 96 * pg:96 * pg + 96],
                start=True, stop=True,
                tile_position=None if gl == 0 else (64, 64),
            )

    # ---- hp = [h2 | h2] per pair (+ b2) ----
    hp_t = gsb.tile([128, GRP * 192], BF16, tag="hp")
    hp = hp_t[:, 0:192 * gs]
    a2_b = a2.rearrange("p (pr t v) -> p pr t v", pr=gs,
                        t=1).to_broadcast([128, gs, 2, 96])
    hpv = hp.rearrange("p (pr t v) -> p pr t v", pr=gs, t=2)
    nc.scalar.activation(out=hpv, in_=a2_b, func=AF.Copy)
    if has_b2:
        hpb_t = gsb.tile([128, GRP * 192], BF16, tag="hpb")
        hpb = hpb_t[:, 0:192 * gs]
        nc.vector.tensor_tensor(
            out=hpb.rearrange("p (pr v) -> p pr v", pr=gs),
            in0=hp.rearrange("p (pr v) -> p pr v", pr=gs),
            in1=b2d[:].rearrange("p (t v) -> p t v", t=1).to_broadcast(
                [128, gs, 192]),
            op=mybir.AluOpType.add)
        hp = hpb

    # ---- conv: units of (gl, oc), 3 tap-matmuls spanning the group ----
    ysb = psb.tile([128, GRP * 768], BF16, tag="ysb")
    ysv = ysb[:, 0:768 * gs].rearrange("p (pr r) -> p pr r", pr=gs)
    for gl in range(2):
        h = slice(64 * gl, 64 * gl + 64)
        for oc in range(4):
            yp = cps.tile([128, 384], FP32, tag="yp")
            ypv = yp[:, 0:96 * gs].rearrange("p (pr v) -> p pr v", pr=gs)
            for k in range(3):
                tap = (95, 0, 1)[k]
                nc.tensor.matmul(
                    ypv,
                    cwd[h, 128 * (4 * k + oc):128 * (4 * k + oc) + 128],
                    hp[64 * gl:64 * gl + 64, :].rearrange(
                        "p (pr v) -> p pr v", pr=gs)[:, :, tap:tap + 96],
                    start=(k == 0), stop=(k == 2),
                )
            co = 384 * gl + 96 * oc
            if (gl + oc) % 2 == 0:
                nc.scalar.activation(out=ysv[:, :, co:co + 96], in_=ypv,
                                     func=AF.Copy)
            else:
                nc.vector.tensor_copy(out=ysv[:, :, co:co + 96], in_=ypv)
    nc.sync.dma_start(
        out=y_d[q0:q0 + gs].rearrange("g p v -> p g v"),
        in_=ysb[:, 0:768 * gs].rearrange("p (g v) -> p g v", g=gs))


def build_gcn_kernel(tc, outs, ins, has_b1=False, has_b2=False):
    nc = tc.nc
    y_d = outs["y"]         # [32, 128, 768] bf16

    from contextlib import ExitStack
    ctx = ExitStack()
    const = ctx.enter_context(tc.tile_pool(name="const", bufs=1))
    ohp = ctx.enter_context(tc.tile_pool(name="ohp", bufs=3))
    gsb = ctx.enter_context(tc.tile_pool(name="gsb", bufs=3))
    psb = ctx.enter_context(tc.tile_pool(name="psb", bufs=4))
    gps = ctx.enter_context(tc.tile_pool(name="gps", bufs=2, space="PSUM"))
    tps = ctx.enter_context(tc.tile_pool(name="tps", bufs=2, space="PSUM"))
    pps = ctx.enter_context(tc.tile_pool(name="pps", bufs=2, space="PSUM"))
    cps = ctx.enter_context(tc.tile_pool(name="cps", bufs=2, space="PSUM"))

    # ---- constants (edge/iota first: they gate the pipeline head) ----
    etr = const.tile([128, 512], BF16)
    nc.sync.dma_start(out=etr[:], in_=ins["etr"][:])
    iota = const.tile([128, 64], BF16)
    nc.sync.dma_start(out=iota[:], in_=ins["iota"][:])
    i64d = const.tile([128, 64], BF16)
    nc.sync.dma_start(out=i64d[:], in_=ins["i64d"][:])
    id1f = const.tile([128, 128], FP32)
    nc.sync.dma_start(out=id1f[:], in_=ins["id1f"][:])
    xt = const.tile([128, 32 * 96], BF16)
    nc.sync.dma_start(out=xt[:], in_=ins["xt"][:])
    w1t = const.tile([96, 512], BF16)
    nc.sync.dma_start(out=w1t[:], in_=ins["w1t"][:])
    w2t = const.tile([128, 384], BF16)
    nc.sync.dma_start(out=w2t[:], in_=ins["w2t"][:])
    cwd = const.tile([128, 1536], BF16)
    nc.sync.dma_start(out=cwd[:], in_=ins["cwd"][:])
    b1c = b2d = None
    if has_b1:
        b1c = const.tile([128, 4], FP32)
        nc.sync.dma_start(out=b1c[:], in_=ins["b1c"][:])
    if has_b2:
        b2d = const.tile([128, 192], BF16)
        nc.sync.dma_start(out=b2d[:], in_=ins["b2d"][:])

    P = (const, ohp, gsb, psb, gps, tps, pps, cps, xt, etr, w1t, w2t, cwd,
         i64d, iota, id1f, b1c, b2d, y_d)
    q0 = 0
    for gs in GROUPS:
        _emit_group(nc, P, q0, gs, has_b1, has_b2)
        q0 += gs

    ctx.close()


# ---------------- host side ----------------

def _prep_consts(W1, b1, W2, b2, conv_w):
    bf = ml_dtypes.bfloat16
    w1t = np.ascontiguousarray(W1.T).astype(bf)                    # [96, 512]
    w2t = np.ascontiguousarray(
        W2.T.reshape(4, 128, 96).transpose(1, 0, 2).reshape(128, 384)
    ).astype(bf)
    # cwd[i, (k, oc, o_lo)] = conv_w[oc*128+o_lo, i, k], duplicated rows
    base = np.ascontiguousarray(
        conv_w.transpose(1, 2, 0).reshape(64, 3 * 4 * 128))
    cwd = np.concatenate([base, base], axis=0).astype(bf)          # [128,1536]
    i64 = np.eye(64)
    i64d = np.concatenate([i64, i64], axis=0).astype(bf)           # [128, 64]
    iota = np.ascontiguousarray(
        np.broadcast_to(np.arange(64).astype(bf), (128, 64)))
    id1f = np.eye(128, dtype=np.float32)
    consts = dict(w1t=w1t, w2t=w2t, cwd=cwd, i64d=i64d, iota=iota, id1f=id1f)
    has_b1 = bool(np.any(b1))
    has_b2 = bool(np.any(b2))
    if has_b1:
        consts["b1c"] = np.ascontiguousarray(
            b1.reshape(4, 128).T).astype(np.float32)
    if has_b2:
        b2d = np.ascontiguousarray(
            np.broadcast_to(np.tile(b2, 2).astype(bf), (128, 192)))
        consts["b2d"] = b2d
    return consts, has_b1, has_b2


_NC_CACHE = {}


def _get_nc(has_b1, has_b2):
    key = (has_b1, has_b2)
    if key in _NC_CACHE:
        return _NC_CACHE[key]
    nc = bacc.Bacc("TRN2", target_bir_lowering=False, debug=False)
    ins = {
        "xt": nc.dram_tensor("xt", [128, 32 * 96], BF16,
                             kind="ExternalInput").ap(),
        "etr": nc.dram_tensor("etr", [128, 512], BF16,
                              kind="ExternalInput").ap(),
        "w1t": nc.dram_tensor("w1t", [96, 512], BF16,
                              kind="ExternalInput").ap(),
        "w2t": nc.dram_tensor("w2t", [128, 384], BF16,
                              kind="ExternalInput").ap(),
        "cwd": nc.dram_tensor("cwd", [128, 1536], BF16,
                              kind="ExternalInput").ap(),
        "i64d": nc.dram_tensor("i64d", [128, 64], BF16,
                               kind="ExternalInput").ap(),
        "iota": nc.dram_tensor("iota", [128, 64], BF16,
                               kind="ExternalInput").ap(),
        "id1f": nc.dram_tensor("id1f", [128, 128], FP32,
                               kind="ExternalInput").ap(),
    }
    if has_b1:
        ins["b1c"] = nc.dram_tensor("b1c", [128, 4], FP32,
                                    kind="ExternalInput").ap()
    if has_b2:
        ins["b2d"] = nc.dram_tensor("b2d", [128, 192], BF16,
                                    kind="ExternalInput").ap()
    outs = {
        "y": nc.dram_tensor("y", [NPAIR, 128, 768], BF16,
                            kind="ExternalOutput").ap(),
    }
    with tile.TileContext(nc) as tc:
        build_gcn_kernel(tc, outs, ins, has_b1, has_b2)
    nc.compile()
    _NC_CACHE[key] = nc
    return nc


def kernel(x, edge_index, W1, b1, W2, b2, conv_w, _trace=False):
    bf = ml_dtypes.bfloat16
    x = np.asarray(x)
    edge_index = np.asarray(edge_index)
    consts, has_b1, has_b2 = _prep_consts(
        np.asarray(W1), np.asarray(b1), np.asarray(W2), np.asarray(b2),
        np.asarray(conv_w))
    nc = _get_nc(has_b1, has_b2)

    in_maps = []
    for cid in range(N_CORES):
        sl = slice(cid * G, (cid + 1) * G)
        m = dict(consts)
        xc = np.asarray(x[sl])                       # [64, 96, 64]
        m["xt"] = np.ascontiguousarray(
            xc.reshape(32, 2, 96, 64).transpose(1, 3, 0, 2).reshape(
                128, 32 * 96)).astype(bf)
        ec = np.asarray(edge_index[sl])              # [64, 2, 512]
        m["etr"] = np.ascontiguousarray(
            ec.reshape(32, 2, 2, 4, 128).transpose(4, 0, 3, 2, 1).reshape(
                128, 512)).astype(bf)
        in_maps.append(m)

    res = run_bass_kernel_spmd(nc, in_maps, core_ids=list(range(N_CORES)),
                               trace=_trace)
    parts = []
    for cid in range(N_CORES):
        arr = np.asarray(res.results[cid]["y"])      # [32, 128, 768] bf16
        yc = arr.reshape(32, 128, 2, 4, 96).transpose(0, 2, 4, 3, 1)
        parts.append(yc.reshape(G, 96, 512).astype(np.float32))
    y = np.concatenate(parts, axis=0)
    if _trace:
        kernel.last_results = res
    return y
